# revision 15
# baseline (speedup 1.0000x reference)
"""HSTU-style attention block (RoPE + multi-scale temporal agg + SDPA + LN + out-proj)
for Trainium2, data-parallel over batch across 8 NeuronCores.

Per-core layout strategy (batch element per core):
  - host pre-transposes X so projections run with activations as lhsT
  - Q/K/V projected into natural [s, h'] layout; RoPE applied in-place in bf16
    (all-bf16 packed operands ride the DVE 2x mode)
  - temporal aggregation applied as a matmul against a host-built [S, S] matrix T
    (softmax(temporal_weights)); band structure (|s'-s| <= 11) trims contraction
    chunks at 256-wide output granularity; Q/K produced transposed, V natural
    with an extra ones column so softmax denominators ride the PV matmul
  - attention computes scores^T per head over the FULL query range: two N=512
    matmuls land in one two-bank [128,1024] PSUM tile so a single scalar-engine
    Exp covers each key chunk (the Exp stream is the phase-2 floor), pipelined
    AHEAD chunks in front of the PV accumulation
  - LayerNorm statistics accumulate on the DVE in bf16 (sum / sum-of-squares
    per chunk); one block of 16 transposed N=1 ones-matmuls at the end of
    phase 2 moves them into the [s-partition] layout, where gamma/beta fold
    into the out-projection weights (host) and the LN apply collapses to a
    per-partition scale at PSUM eviction plus a rank-1 mu*rstd correction
All matmuls run in bfloat16 (fp32 PSUM accumulation).
"""

import numpy as np
import ml_dtypes
import concourse.mybir as mybir
import concourse.tile as tile
from concourse import bacc
from concourse.bass_utils import run_bass_kernel_spmd

B, S, H, NH = 8, 1024, 1024, 16
HD = H // NH  # 64
P = 128
SO = S // P  # 8
HO = H // P  # 8
N_SCALES = 4
LN_EPS = 1e-5
F32 = mybir.dt.float32
BF16 = mybir.dt.bfloat16
NPBF16 = ml_dtypes.bfloat16

N_CORES = 8
AHEAD = 2  # exp pipeline depth (score chunks ahead of PV)


# ---------------------------------------------------------------- host helpers
def _softmax_np(x):
    x = np.asarray(x, np.float64)
    e = np.exp(x - x.max())
    return e / e.sum()


def _temporal_matrix(temporal_weights):
    """[S, S] matrix T with (T @ x) == temporal_agg(x) along the sequence axis."""
    w = _softmax_np(temporal_weights)
    T = np.eye(S, dtype=np.float64) * w[0]
    for scale in range(1, N_SCALES):
        p = max(1, S // (2 ** scale))
        k = S // p
        pool = np.zeros((p, S), dtype=np.float64)
        for j in range(p):
            pool[j, j * k:(j + 1) * k] = 1.0 / k
        coord = (np.arange(S, dtype=np.float64) + 0.5) * (p / S) - 0.5
        coord = np.clip(coord, 0.0, None)
        i0 = np.minimum(np.floor(coord).astype(np.int64), p - 1)
        i1 = np.minimum(i0 + 1, p - 1)
        lam = (coord - i0).astype(np.float32).astype(np.float64)
        interp = np.zeros((S, p), dtype=np.float64)
        interp[np.arange(S), i0] += 1.0 - lam
        interp[np.arange(S), i1] += lam
        T += w[scale] * (interp @ pool)
    return T.astype(np.float32)


def _rope_tables():
    inv_freq = 1.0 / (10000.0 ** (np.arange(0, HD, 2, dtype=np.float64) / HD))
    freqs = np.arange(S, dtype=np.float64)[:, None] * inv_freq[None, :]
    cos = np.repeat(np.cos(freqs), 2, axis=-1).astype(np.float32)  # [S, HD]
    sin = np.repeat(np.sin(freqs), 2, axis=-1).astype(np.float32)
    return cos, sin


def _nat(x):
    """[S, D] -> [P, S//P, D] with x[so*P+p, d] = out[p, so, d]."""
    return np.ascontiguousarray(x.reshape(SO, P, x.shape[-1]).transpose(1, 0, 2))


def _xt_chunks(x):
    """[S, H] -> [P, SO, HO*P] with out[p, so, ho*P + i] = x[so*P + i, ho*P + p]."""
    return np.ascontiguousarray(
        x.reshape(SO, P, HO, P).transpose(3, 0, 2, 1).reshape(P, SO, H))


# ---------------------------------------------------------------- bass program
def _build_program():
    nc = bacc.Bacc("TRN2", target_bir_lowering=False, debug=False)

    d_xt = {a: nc.dram_tensor(f"xt_{a}", [P, SO, H], BF16, kind="ExternalInput")
            for a in ("v", "q", "k")}
    d_w = {a: nc.dram_tensor(f"w_{a}", [P, HO, H], BF16, kind="ExternalInput")
           for a in ("v", "q", "k", "o")}
    d_b = {a: nc.dram_tensor(f"b_{a}", [1, H], F32, kind="ExternalInput")
           for a in ("v", "q", "k", "o")}
    d_g1n = nc.dram_tensor("g1n", [1, H], F32, kind="ExternalInput")
    d_tt = nc.dram_tensor("tt", [P, SO, S], BF16, kind="ExternalInput")
    d_cos = nc.dram_tensor("cos_t", [P, SO, HD], BF16, kind="ExternalInput")
    d_sin = nc.dram_tensor("sin_t", [P, SO, HD], BF16, kind="ExternalInput")
    d_y = nc.dram_tensor("y", [P, SO, H], F32, kind="ExternalOutput")
    d_zp = nc.dram_tensor("zpad", [HD, S], BF16, kind="ExternalInput")
    # per-chunk scratch so a head's reload only waits on its own spill DMA
    d_qs = [nc.dram_tensor(f"q_scr{hc}", [P, S], BF16) for hc in range(HO)]
    d_ks = [nc.dram_tensor(f"k_scr{hc}", [P, S], BF16) for hc in range(HO)]

    with tile.TileContext(nc) as tc:
        with (
            tc.tile_pool(name="const", bufs=1) as cpool,
            tc.tile_pool(name="big", bufs=5) as big,
            tc.tile_pool(name="s4", bufs=6) as s4,
            tc.tile_pool(name="xt", bufs=3) as xtp,
            tc.tile_pool(name="rot", bufs=2) as rotp,
            tc.tile_pool(name="kq", bufs=6) as kqp,
            tc.tile_pool(name="s2", bufs=6) as s2,
            tc.tile_pool(name="et", bufs=4) as etp,
            tc.tile_pool(name="sq", bufs=2) as sqp,
            tc.tile_pool(name="at", bufs=8) as atp,
            tc.tile_pool(name="acc", bufs=2) as accp,
            tc.tile_pool(name="mm_ps", bufs=2, space="PSUM") as mmps,
            tc.tile_pool(name="sc_ps", bufs=2, space="PSUM") as scps,
            tc.tile_pool(name="pv_ps", bufs=2, space="PSUM") as pvps,
        ):
            cos_t = cpool.tile([P, SO, HD], BF16, name="cos_t")
            sin_t = cpool.tile([P, SO, HD], BF16, name="sin_t")
            nc.sync.dma_start(cos_t[:], d_cos.ap())
            nc.sync.dma_start(sin_t[:], d_sin.ap())
            ones = cpool.tile([P, 1], F32, name="ones")
            nc.vector.memset(ones[:], 1.0)
            eps_t = cpool.tile([P, 1], F32, name="eps_t")
            nc.vector.memset(eps_t[:], LN_EPS)
            # stats rhs: 1/H so the PSUM accumulators hold means directly
            ones_st = cpool.tile([P, 1], BF16, name="ones_st")
            nc.vector.memset(ones_st[:], 1.0 / H)

            def _rope_chunk(a_nat, so):
                """In-place bf16 RoPE on a_nat[:, so, :] (DVE 2x mode)."""
                ch = a_nat[:, so, :]
                ch3 = ch.rearrange("p (nh d) -> p nh d", d=HD)
                ch4 = ch.rearrange("p (nh hf dd) -> p nh hf dd", hf=2, dd=HD // 2)
                rot = rotp.tile([P, H], BF16, tag="rot")
                rot4 = rot[:].rearrange("p (nh hf dd) -> p nh hf dd",
                                        hf=2, dd=HD // 2)
                rot3 = rot[:].rearrange("p (nh d) -> p nh d", d=HD)
                nc.vector.tensor_scalar_mul(rot4[:, :, 0, :], ch4[:, :, 1, :], -1.0)
                nc.vector.tensor_copy(rot4[:, :, 1, :], ch4[:, :, 0, :])
                cb = cos_t[:, so, :][:, None, :].to_broadcast((P, NH, HD))
                sb = sin_t[:, so, :][:, None, :].to_broadcast((P, NH, HD))
                nc.vector.tensor_tensor(ch3[:], ch3[:], cb, mybir.AluOpType.mult)
                nc.vector.tensor_tensor(rot3[:], rot3[:], sb, mybir.AluOpType.mult)
                nc.vector.tensor_tensor(ch[:], ch[:], rot[:], mybir.AluOpType.add)

            def project(a, do_rope=False):
                """A_nat [P, SO, H] (bf16) = X @ W_a + b_a, optional fused RoPE."""
                w_t = big.tile([P, HO, H], BF16, tag="big")
                nc.sync.dma_start(w_t[:], d_w[a].ap())
                brow = s4.tile([1, H], F32, tag="s4")
                nc.sync.dma_start(brow[:], d_b[a].ap())
                bb = s4.tile([P, H], F32, tag="s4")
                nc.gpsimd.partition_broadcast(bb[:], brow[:])
                a_nat = big.tile([P, SO, H], BF16, tag="big")
                for so in range(SO):
                    xt_c = xtp.tile([P, HO, P], BF16, tag="xt")
                    nc.sync.dma_start(xt_c[:], d_xt[a].ap()[:, so, :])
                    for nh in range(2):
                        ps = mmps.tile([P, 512], F32, tag="mm")
                        for ko in range(HO):
                            nc.tensor.matmul(
                                ps[:], xt_c[:, ko, :],
                                w_t[:, ko, nh * 512:(nh + 1) * 512],
                                start=(ko == 0), stop=(ko == HO - 1))
                        nc.vector.tensor_tensor(
                            a_nat[:, so, nh * 512:(nh + 1) * 512], ps[:],
                            bb[:, nh * 512:(nh + 1) * 512], mybir.AluOpType.add)
                    if do_rope:
                        _rope_chunk(a_nat, so)
                return a_nat

            BAND = 12  # T[s', s] == 0 for |s' - s| > 11 (structural)

            def _band_sos(o0, o1):
                """so chunks whose s-range intersects [o0-BAND, o1+BAND)."""
                return [so for so in range(SO)
                        if so * P + P > o0 - BAND and so * P < o1 + BAND]

            def spill_chunk(a_nat, tt, hc, d_scr):
                """One h'-chunk of (T @ A).T spilled to DRAM scratch. Runs
                interleaved with attention, so the eviction rides the DVE
                (the scalar engine is saturated by the Exp stream there)."""
                for sh in range(2):
                    ps = mmps.tile([P, 512], F32, tag="mm")
                    for q in range(2):
                        o0 = sh * 512 + q * 256
                        sos = _band_sos(o0, o0 + 256)
                        for so in sos:
                            nc.tensor.matmul(
                                ps[:, q * 256:(q + 1) * 256],
                                a_nat[:, so, hc * P:(hc + 1) * P],
                                tt[:, so, o0:o0 + 256],
                                start=(so == sos[0]), stop=(so == sos[-1]),
                                skip_group_check=True)
                    ev = s2.tile([P, 512], BF16, tag="s2")
                    nc.vector.tensor_copy(ev[:], ps[:])
                    nc.sync.dma_start(
                        d_scr[hc].ap()[:, sh * 512:(sh + 1) * 512], ev[:])

            def t_agg_v(v_nat, tt):
                """V_ext [P, SO, NH, HD+1] (bf16) = T @ V with ones column."""
                v_ext = big.tile([P, SO, NH, HD + 1], BF16, tag="big")
                nc.vector.tensor_copy(
                    v_ext[:, :, :, HD:HD + 1],
                    ones[:, None, None, :].to_broadcast((P, SO, NH, 1)))
                for sc in range(SO):
                    sos = _band_sos(sc * P, (sc + 1) * P)
                    for dh in range(2):
                        ps = mmps.tile([P, 512], F32, tag="mm")
                        for so in sos:
                            nc.tensor.matmul(
                                ps[:], tt[:, so, sc * P:(sc + 1) * P],
                                v_nat[:, so, dh * 512:(dh + 1) * 512],
                                start=(so == sos[0]), stop=(so == sos[-1]))
                        pvw = ps[:].rearrange("p (nh d) -> p nh d", d=HD)
                        nc.scalar.copy(
                            v_ext[:, sc, dh * 8:(dh + 1) * 8, 0:HD], pvw)
                return v_ext

            # ---- phase 1: V, Q, K  (projection + RoPE + temporal aggregation)
            v_nat = project("v")
            tt = big.tile([P, SO, S], BF16, tag="big")
            nc.sync.dma_start(tt[:], d_tt.ap())
            v_ext = t_agg_v(v_nat, tt)

            q_nat = project("q", do_rope=True)
            k_nat = project("k", do_rope=True)

            pre_kq = {}

            def _load_head(h):
                off = (h % 2) * HD
                hc = h // 2
                kh = kqp.tile([P, S], BF16, tag="kq", name=f"kh{h}")
                nc.sync.dma_start(kh[0:HD, :], d_ks[hc].ap()[off:off + HD, :])
                nc.sync.dma_start(kh[HD:P, :], d_zp.ap())
                qh = kqp.tile([P, S], BF16, tag="kq", name=f"qh{h}")
                nc.sync.dma_start(qh[0:HD, :], d_qs[hc].ap()[off:off + HD, :])
                nc.sync.dma_start(qh[HD:P, :], d_zp.ap())
                return kh, qh

            # spill the first two h'-chunks of Q/K aggregation upfront; the
            # remaining chunks interleave with the head loop two chunks ahead
            for hc0 in (0, 1):
                spill_chunk(q_nat, tt, hc0, d_qs)
                spill_chunk(k_nat, tt, hc0, d_ks)
            pre_kq[0] = _load_head(0)
            pre_kq[1] = _load_head(1)

            # prefetch out-projection weights + folded LN rows during phase 2
            wo_t = big.tile([P, HO, H], BF16, tag="big")
            nc.sync.dma_start(wo_t[:], d_w["o"].ap())
            b1r = s4.tile([1, H], F32, tag="s4")
            nc.sync.dma_start(b1r[:], d_b["o"].ap())
            b1b = cpool.tile([P, H], F32, name="b1b")
            nc.gpsimd.partition_broadcast(b1b[:], b1r[:])
            g1r = s4.tile([1, H], F32, tag="s4")
            nc.sync.dma_start(g1r[:], d_g1n.ap())
            g1nb = cpool.tile([P, H], F32, name="g1nb")
            nc.gpsimd.partition_broadcast(g1nb[:], g1r[:])

            # ---- phase 2: attention. One attention-out tile per h' chunk
            # (separate tensors keep the scheduler from inventing cross-chunk
            # dependencies); LN stats accumulate on the DVE only.
            attn_c = [atp.tile([P, S], BF16, tag="at", name=f"attn{c}")
                      for c in range(HO)]
            acc = accp.tile([P, S], BF16, tag="acc", name="acc")
            acc2 = accp.tile([P, S], BF16, tag="acc", name="acc2")

            rb_c = None
            for h in range(NH):
                hc, off = h // 2, (h % 2) * HD
                # zero-pad the contraction dim to K=128 (rows 64:128 from a
                # DRAM zeros pad) to keep the PE activity profile flat
                if h in pre_kq:
                    kh, qh = pre_kq[h]
                else:
                    kh, qh = _load_head(h)
                if off == 0:
                    if hc + 2 < HO:
                        spill_chunk(q_nat, tt, hc + 2, d_qs)
                        spill_chunk(k_nat, tt, hc + 2, d_ks)
                    rb_c = s4.tile([P, S], F32, tag="s4")
                pv_a = pvps.tile([P, 512], F32, tag="pv", name="pv_a")
                pv_b = pvps.tile([P, 512], F32, tag="pv", name="pv_b")
                # software-pipelined: both query halves of a key chunk land in
                # one two-bank PSUM tile so a single Exp serves the chunk,
                # running AHEAD chunks in front of the PV accumulation
                ets = []
                for kc in range(SO + AHEAD):
                    if kc < SO:
                        sp = scps.tile([P, 1024], F32, tag="sc")
                        nc.tensor.matmul(
                            sp[:, 0:512], kh[0:P, kc * P:(kc + 1) * P],
                            qh[0:P, 0:512],
                            start=True, stop=True, skip_group_check=True)
                        nc.tensor.matmul(
                            sp[:, 512:1024], kh[0:P, kc * P:(kc + 1) * P],
                            qh[0:P, 512:1024],
                            start=True, stop=True, skip_group_check=True)
                        e_t = etp.tile([P, 1024], BF16, tag="et")
                        nc.scalar.activation(
                            e_t[:], sp[:],
                            mybir.ActivationFunctionType.Exp, scale=0.125)
                        ets.append(e_t)
                    if kc >= AHEAD:
                        j = kc - AHEAD
                        nc.tensor.matmul(
                            pv_a[0:HD + 1, :], v_ext[:, j, h, :],
                            ets[j][:, 0:512],
                            start=(j == 0), stop=(j == SO - 1),
                            skip_group_check=True)
                        nc.tensor.matmul(
                            pv_b[0:HD + 1, :], v_ext[:, j, h, :],
                            ets[j][:, 512:1024],
                            start=(j == 0), stop=(j == SO - 1),
                            skip_group_check=True)
                # evict raw out + sums; broadcast sums (no PE dependency).
                # partition_broadcast only writes reliably at partition 0,
                # so odd heads bounce through a temp + DVE copy.
                for q2, pv in ((0, pv_a), (1, pv_b)):
                    qs = slice(q2 * 512, (q2 + 1) * 512)
                    nc.vector.tensor_copy(attn_c[hc][off:off + HD, qs],
                                          pv[0:HD, :])
                    srow = s2.tile([1, 512], F32, tag="s2")
                    nc.vector.tensor_copy(srow[:], pv[HD:HD + 1, :])
                    if off == 0:
                        nc.gpsimd.partition_broadcast(rb_c[0:HD, qs], srow[:])
                    else:
                        tmp = s2.tile([HD, 512], F32, tag="s2")
                        nc.gpsimd.partition_broadcast(tmp[:], srow[:])
                        nc.vector.tensor_copy(rb_c[off:off + HD, qs], tmp[:])
                if off == HD:
                    # chunk hc complete: normalize + LN stats (all DVE)
                    rcp_c = s4.tile([P, S], F32, tag="s4")
                    nc.vector.reciprocal_approx_fast(rcp_c[:], rb_c[:])
                    nc.vector.tensor_tensor(attn_c[hc][:], attn_c[hc][:],
                                            rcp_c[:], mybir.AluOpType.mult)
                    sq_c = sqp.tile([P, S], BF16, tag="sq")
                    nc.vector.tensor_tensor(sq_c[:], attn_c[hc][:],
                                            attn_c[hc][:],
                                            mybir.AluOpType.mult)
                    if hc == 0:
                        nc.vector.tensor_copy(acc[:], attn_c[0][:])
                        nc.vector.tensor_copy(acc2[:], sq_c[:])
                    else:
                        nc.vector.tensor_tensor(acc[:], acc[:], attn_c[hc][:],
                                                mybir.AluOpType.add)
                        nc.vector.tensor_tensor(acc2[:], acc2[:], sq_c[:],
                                                mybir.AluOpType.add)

            # ---- phase 3: one block of transposed stats matmuls ([s-part,
            # so] layout) + LN scale factors -- all tiny ops
            stat_t = pvps.tile([P, 16], F32, tag="pv", name="stat_t")
            for so in range(SO):
                nc.tensor.matmul(
                    stat_t[:, so:so + 1], acc[:, so * P:(so + 1) * P],
                    ones_st[:], start=True, stop=True, skip_group_check=True)
                nc.tensor.matmul(
                    stat_t[:, 8 + so:9 + so], acc2[:, so * P:(so + 1) * P],
                    ones_st[:], start=True, stop=True, skip_group_check=True)
            m2 = s2.tile([P, 8], F32, tag="s2")
            nc.scalar.square(m2[:], stat_t[:, 0:8])
            var_t = s2.tile([P, 8], F32, tag="s2")
            nc.vector.tensor_tensor(var_t[:], stat_t[:, 8:16], m2[:],
                                    mybir.AluOpType.subtract)
            nc.scalar.activation(var_t[:], var_t[:],
                                 mybir.ActivationFunctionType.Sqrt, bias=eps_t[:])
            rstd = s2.tile([P, 8], F32, tag="s2")
            nc.vector.reciprocal_approx_fast(rstd[:], var_t[:])
            rmu = s2.tile([P, 8], F32, tag="s2")
            nc.vector.tensor_tensor(rmu[:], stat_t[:, 0:8], rstd[:],
                                    mybir.AluOpType.mult)

            # ---- phase 4: output projection on raw attn with fused LN:
            # y = rstd*(attn^T @ W') - (mu*rstd)*g1 + b1
            for so in range(SO):
                for nh in range(2):
                    t2 = s2.tile([P, 512], F32, tag="s2")
                    nc.vector.scalar_tensor_tensor(
                        t2[:], g1nb[:, nh * 512:(nh + 1) * 512],
                        rmu[:, so:so + 1], b1b[:, nh * 512:(nh + 1) * 512],
                        mybir.AluOpType.mult, mybir.AluOpType.add)
                    ps = mmps.tile([P, 512], F32, tag="mm")
                    for hc in range(HO):
                        nc.tensor.matmul(
                            ps[:], attn_c[hc][:, so * P:(so + 1) * P],
                            wo_t[:, hc, nh * 512:(nh + 1) * 512],
                            start=(hc == 0), stop=(hc == HO - 1))
                    ych = s2.tile([P, 512], F32, tag="s2")
                    nc.scalar.activation(ych[:], ps[:],
                                         mybir.ActivationFunctionType.Copy,
                                         scale=rstd[:, so:so + 1])
                    nc.vector.tensor_tensor(ych[:], ych[:], t2[:],
                                            mybir.AluOpType.add)
                    nc.sync.dma_start(
                        d_y.ap()[:, so, nh * 512:(nh + 1) * 512], ych[:])

    nc.compile()
    return nc


_NC = None


def _get_nc():
    global _NC
    if _NC is None:
        _NC = _build_program()
    return _NC


def _host_inputs(query, key, value, Wq, bq, Wk, bk, Wv, bv, Wo, bo,
                 temporal_weights, ln_gamma, ln_beta):
    T = _temporal_matrix(temporal_weights)
    tt_host = np.ascontiguousarray(  # TT[p, so, s'] = T[s', so*P+p]
        T.T.reshape(SO, P, S).transpose(1, 0, 2)).astype(NPBF16)
    cos, sin = _rope_tables()
    gam = np.asarray(ln_gamma, np.float32)
    bet = np.asarray(ln_beta, np.float32)
    Wo32 = np.asarray(Wo, np.float32)
    Wfold = gam[:, None] * Wo32               # gamma folded into out-proj
    g1n = -(gam @ Wo32).reshape(1, H)         # rank-1 LN correction row
    b1 = (bet @ Wo32 + np.asarray(bo, np.float32)).reshape(1, H)
    common = {
        "w_v": _nat(np.asarray(Wv, np.float32)).astype(NPBF16),
        "w_q": _nat(np.asarray(Wq, np.float32)).astype(NPBF16),
        "w_k": _nat(np.asarray(Wk, np.float32)).astype(NPBF16),
        "w_o": _nat(Wfold).astype(NPBF16),
        "b_v": np.asarray(bv, np.float32).reshape(1, H),
        "b_q": np.asarray(bq, np.float32).reshape(1, H),
        "b_k": np.asarray(bk, np.float32).reshape(1, H),
        "b_o": b1,
        "g1n": g1n,
        "tt": tt_host,
        "zpad": np.zeros((HD, S), NPBF16),
        "cos_t": _nat(cos).astype(NPBF16),
        "sin_t": _nat(sin).astype(NPBF16),
    }
    in_maps = []
    for c in range(N_CORES):
        m = dict(common)
        m["xt_q"] = _xt_chunks(np.asarray(query[c], np.float32)).astype(NPBF16)
        m["xt_k"] = _xt_chunks(np.asarray(key[c], np.float32)).astype(NPBF16)
        m["xt_v"] = _xt_chunks(np.asarray(value[c], np.float32)).astype(NPBF16)
        in_maps.append(m)
    return in_maps


def kernel(query, key, value, Wq, bq, Wk, bk, Wv, bv, Wo, bo,
           temporal_weights, ln_gamma, ln_beta):
    in_maps = _host_inputs(query, key, value, Wq, bq, Wk, bk, Wv, bv, Wo, bo,
                           temporal_weights, ln_gamma, ln_beta)
    nc = _get_nc()
    res = run_bass_kernel_spmd(nc, in_maps, list(range(N_CORES)))
    out = np.empty((B, S, H), np.float32)
    for c in range(N_CORES):
        y = res.results[c]["y"]  # [P, SO, H]
        out[c] = y.transpose(1, 0, 2).reshape(S, H)
    return out


# revision 16
# speedup vs baseline: 1.0157x; 1.0157x over previous
"""HSTU-style attention block (RoPE + multi-scale temporal agg + SDPA + LN + out-proj)
for Trainium2, data-parallel over batch across 8 NeuronCores.

Per-core layout strategy (batch element per core):
  - host pre-transposes X so projections run with activations as lhsT
  - Q/K/V projected into natural [s, h'] layout; RoPE applied in-place in bf16
    (all-bf16 packed operands ride the DVE 2x mode)
  - temporal aggregation applied as a matmul against a host-built [S, S] matrix T
    (softmax(temporal_weights)); band structure (|s'-s| <= 11) trims contraction
    chunks at 256-wide output granularity; Q/K produced transposed, V natural
    with an extra ones column so softmax denominators ride the PV matmul
  - attention computes scores^T per head over the FULL query range: two N=512
    matmuls land in one two-bank [128,1024] PSUM tile so a single scalar-engine
    Exp covers each key chunk (the Exp stream is the phase-2 floor), pipelined
    AHEAD chunks in front of the PV accumulation
  - LayerNorm statistics accumulate on the DVE in bf16 (sum / sum-of-squares
    per chunk); one block of 16 transposed N=1 ones-matmuls at the end of
    phase 2 moves them into the [s-partition] layout, where gamma/beta fold
    into the out-projection weights (host) and the LN apply collapses to a
    per-partition scale at PSUM eviction plus a rank-1 mu*rstd correction
All matmuls run in bfloat16 (fp32 PSUM accumulation).
"""

import numpy as np
import ml_dtypes
import concourse.mybir as mybir
import concourse.tile as tile
from concourse import bacc
from concourse.bass_utils import run_bass_kernel_spmd

B, S, H, NH = 8, 1024, 1024, 16
HD = H // NH  # 64
P = 128
SO = S // P  # 8
HO = H // P  # 8
N_SCALES = 4
LN_EPS = 1e-5
F32 = mybir.dt.float32
BF16 = mybir.dt.bfloat16
NPBF16 = ml_dtypes.bfloat16

N_CORES = 8
AHEAD = 2  # exp pipeline depth (score chunks ahead of PV)


# ---------------------------------------------------------------- host helpers
def _softmax_np(x):
    x = np.asarray(x, np.float64)
    e = np.exp(x - x.max())
    return e / e.sum()


def _temporal_matrix(temporal_weights):
    """[S, S] matrix T with (T @ x) == temporal_agg(x) along the sequence axis."""
    w = _softmax_np(temporal_weights)
    T = np.eye(S, dtype=np.float64) * w[0]
    for scale in range(1, N_SCALES):
        p = max(1, S // (2 ** scale))
        k = S // p
        pool = np.zeros((p, S), dtype=np.float64)
        for j in range(p):
            pool[j, j * k:(j + 1) * k] = 1.0 / k
        coord = (np.arange(S, dtype=np.float64) + 0.5) * (p / S) - 0.5
        coord = np.clip(coord, 0.0, None)
        i0 = np.minimum(np.floor(coord).astype(np.int64), p - 1)
        i1 = np.minimum(i0 + 1, p - 1)
        lam = (coord - i0).astype(np.float32).astype(np.float64)
        interp = np.zeros((S, p), dtype=np.float64)
        interp[np.arange(S), i0] += 1.0 - lam
        interp[np.arange(S), i1] += lam
        T += w[scale] * (interp @ pool)
    return T.astype(np.float32)


def _rope_tables():
    inv_freq = 1.0 / (10000.0 ** (np.arange(0, HD, 2, dtype=np.float64) / HD))
    freqs = np.arange(S, dtype=np.float64)[:, None] * inv_freq[None, :]
    cos = np.repeat(np.cos(freqs), 2, axis=-1).astype(np.float32)  # [S, HD]
    sin = np.repeat(np.sin(freqs), 2, axis=-1).astype(np.float32)
    return cos, sin


def _nat(x):
    """[S, D] -> [P, S//P, D] with x[so*P+p, d] = out[p, so, d]."""
    return np.ascontiguousarray(x.reshape(SO, P, x.shape[-1]).transpose(1, 0, 2))


def _xt_chunks(x):
    """[S, H] -> [P, SO, HO*P] with out[p, so, ho*P + i] = x[so*P + i, ho*P + p]."""
    return np.ascontiguousarray(
        x.reshape(SO, P, HO, P).transpose(3, 0, 2, 1).reshape(P, SO, H))


# ---------------------------------------------------------------- bass program
def _build_program():
    nc = bacc.Bacc("TRN2", target_bir_lowering=False, debug=False)

    d_xt = {a: nc.dram_tensor(f"xt_{a}", [P, SO, H], BF16, kind="ExternalInput")
            for a in ("v", "q", "k")}
    d_w = {a: nc.dram_tensor(f"w_{a}", [P, HO, H], BF16, kind="ExternalInput")
           for a in ("v", "q", "k", "o")}
    d_b = {a: nc.dram_tensor(f"b_{a}", [1, H], F32, kind="ExternalInput")
           for a in ("v", "q", "k", "o")}
    d_g1n = nc.dram_tensor("g1n", [1, H], F32, kind="ExternalInput")
    d_tt = nc.dram_tensor("tt", [P, SO, S], BF16, kind="ExternalInput")
    d_cos = nc.dram_tensor("cos_t", [P, SO, HD], BF16, kind="ExternalInput")
    d_sin = nc.dram_tensor("sin_t", [P, SO, HD], BF16, kind="ExternalInput")
    d_y = nc.dram_tensor("y", [P, SO, H], F32, kind="ExternalOutput")
    d_zp = nc.dram_tensor("zpad", [HD, S], BF16, kind="ExternalInput")
    # per-chunk scratch so a head's reload only waits on its own spill DMA
    d_qs = [nc.dram_tensor(f"q_scr{hc}", [P, S], BF16) for hc in range(HO)]
    d_ks = [nc.dram_tensor(f"k_scr{hc}", [P, S], BF16) for hc in range(HO)]

    with tile.TileContext(nc) as tc:
        with (
            tc.tile_pool(name="const", bufs=1) as cpool,
            tc.tile_pool(name="big", bufs=5) as big,
            tc.tile_pool(name="s4", bufs=6) as s4,
            tc.tile_pool(name="xt", bufs=3) as xtp,
            tc.tile_pool(name="rot", bufs=2) as rotp,
            tc.tile_pool(name="kq", bufs=6) as kqp,
            tc.tile_pool(name="s2", bufs=6) as s2,
            tc.tile_pool(name="et", bufs=4) as etp,
            tc.tile_pool(name="sq", bufs=2) as sqp,
            tc.tile_pool(name="at", bufs=8) as atp,
            tc.tile_pool(name="acc", bufs=2) as accp,
            tc.tile_pool(name="mm_ps", bufs=2, space="PSUM") as mmps,
            tc.tile_pool(name="sc_ps", bufs=2, space="PSUM") as scps,
            tc.tile_pool(name="pv_ps", bufs=2, space="PSUM") as pvps,
        ):
            cos_t = cpool.tile([P, SO, HD], BF16, name="cos_t")
            sin_t = cpool.tile([P, SO, HD], BF16, name="sin_t")
            nc.sync.dma_start(cos_t[:], d_cos.ap())
            nc.sync.dma_start(sin_t[:], d_sin.ap())
            ones = cpool.tile([P, 1], F32, name="ones")
            nc.vector.memset(ones[:], 1.0)
            eps_t = cpool.tile([P, 1], F32, name="eps_t")
            nc.vector.memset(eps_t[:], LN_EPS)
            # stats rhs: 1/H so the PSUM accumulators hold means directly
            ones_st = cpool.tile([P, 1], BF16, name="ones_st")
            nc.vector.memset(ones_st[:], 1.0 / H)

            def _rope_chunk(a_nat, so):
                """In-place bf16 RoPE on a_nat[:, so, :] (DVE 2x mode)."""
                ch = a_nat[:, so, :]
                ch3 = ch.rearrange("p (nh d) -> p nh d", d=HD)
                ch4 = ch.rearrange("p (nh hf dd) -> p nh hf dd", hf=2, dd=HD // 2)
                rot = rotp.tile([P, H], BF16, tag="rot")
                rot4 = rot[:].rearrange("p (nh hf dd) -> p nh hf dd",
                                        hf=2, dd=HD // 2)
                rot3 = rot[:].rearrange("p (nh d) -> p nh d", d=HD)
                nc.vector.tensor_scalar_mul(rot4[:, :, 0, :], ch4[:, :, 1, :], -1.0)
                nc.vector.tensor_copy(rot4[:, :, 1, :], ch4[:, :, 0, :])
                cb = cos_t[:, so, :][:, None, :].to_broadcast((P, NH, HD))
                sb = sin_t[:, so, :][:, None, :].to_broadcast((P, NH, HD))
                nc.vector.tensor_tensor(ch3[:], ch3[:], cb, mybir.AluOpType.mult)
                nc.vector.tensor_tensor(rot3[:], rot3[:], sb, mybir.AluOpType.mult)
                nc.vector.tensor_tensor(ch[:], ch[:], rot[:], mybir.AluOpType.add)

            def project(a, do_rope=False):
                """A_nat [P, SO, H] (bf16) = X @ W_a + b_a, optional fused RoPE."""
                w_t = big.tile([P, HO, H], BF16, tag="big")
                for ko in range(HO):
                    nc.sync.dma_start(w_t[:, ko, :], d_w[a].ap()[:, ko, :])
                brow = s4.tile([1, H], F32, tag="s4")
                nc.sync.dma_start(brow[:], d_b[a].ap())
                bb = s4.tile([P, H], F32, tag="s4")
                nc.gpsimd.partition_broadcast(bb[:], brow[:])
                a_nat = big.tile([P, SO, H], BF16, tag="big")
                for so in range(SO):
                    xt_c = xtp.tile([P, HO, P], BF16, tag="xt")
                    nc.sync.dma_start(xt_c[:], d_xt[a].ap()[:, so, :])
                    for nh in range(2):
                        ps = mmps.tile([P, 512], F32, tag="mm")
                        for ko in range(HO):
                            nc.tensor.matmul(
                                ps[:], xt_c[:, ko, :],
                                w_t[:, ko, nh * 512:(nh + 1) * 512],
                                start=(ko == 0), stop=(ko == HO - 1))
                        nc.vector.tensor_tensor(
                            a_nat[:, so, nh * 512:(nh + 1) * 512], ps[:],
                            bb[:, nh * 512:(nh + 1) * 512], mybir.AluOpType.add)
                    if do_rope:
                        _rope_chunk(a_nat, so)
                return a_nat

            BAND = 12  # T[s', s] == 0 for |s' - s| > 11 (structural)

            def _band_sos(o0, o1):
                """so chunks whose s-range intersects [o0-BAND, o1+BAND)."""
                return [so for so in range(SO)
                        if so * P + P > o0 - BAND and so * P < o1 + BAND]

            def spill_chunk(a_nat, tt, hc, d_scr):
                """One h'-chunk of (T @ A).T spilled to DRAM scratch. Runs
                interleaved with attention, so the eviction rides the DVE
                (the scalar engine is saturated by the Exp stream there)."""
                for sh in range(2):
                    ps = mmps.tile([P, 512], F32, tag="mm")
                    for q in range(2):
                        o0 = sh * 512 + q * 256
                        sos = _band_sos(o0, o0 + 256)
                        for so in sos:
                            nc.tensor.matmul(
                                ps[:, q * 256:(q + 1) * 256],
                                a_nat[:, so, hc * P:(hc + 1) * P],
                                tt[:, so, o0:o0 + 256],
                                start=(so == sos[0]), stop=(so == sos[-1]),
                                skip_group_check=True)
                    ev = s2.tile([P, 512], BF16, tag="s2")
                    nc.vector.tensor_copy(ev[:], ps[:])
                    nc.sync.dma_start(
                        d_scr[hc].ap()[:, sh * 512:(sh + 1) * 512], ev[:])

            def t_agg_v(v_nat, tt):
                """V_ext [P, SO, NH, HD+1] (bf16) = T @ V with ones column."""
                v_ext = big.tile([P, SO, NH, HD + 1], BF16, tag="big")
                nc.vector.tensor_copy(
                    v_ext[:, :, :, HD:HD + 1],
                    ones[:, None, None, :].to_broadcast((P, SO, NH, 1)))
                for sc in range(SO):
                    sos = _band_sos(sc * P, (sc + 1) * P)
                    for dh in range(2):
                        ps = mmps.tile([P, 512], F32, tag="mm")
                        for so in sos:
                            nc.tensor.matmul(
                                ps[:], tt[:, so, sc * P:(sc + 1) * P],
                                v_nat[:, so, dh * 512:(dh + 1) * 512],
                                start=(so == sos[0]), stop=(so == sos[-1]))
                        pvw = ps[:].rearrange("p (nh d) -> p nh d", d=HD)
                        nc.scalar.copy(
                            v_ext[:, sc, dh * 8:(dh + 1) * 8, 0:HD], pvw)
                return v_ext

            # ---- phase 1: V, Q, K  (projection + RoPE + temporal aggregation)
            v_nat = project("v")
            tt = big.tile([P, SO, S], BF16, tag="big")
            nc.sync.dma_start(tt[:], d_tt.ap())
            v_ext = t_agg_v(v_nat, tt)

            q_nat = project("q", do_rope=True)
            k_nat = project("k", do_rope=True)

            pre_kq = {}

            def _load_head(h):
                off = (h % 2) * HD
                hc = h // 2
                kh = kqp.tile([P, S], BF16, tag="kq", name=f"kh{h}")
                nc.sync.dma_start(kh[0:HD, :], d_ks[hc].ap()[off:off + HD, :])
                nc.sync.dma_start(kh[HD:P, :], d_zp.ap())
                qh = kqp.tile([P, S], BF16, tag="kq", name=f"qh{h}")
                nc.sync.dma_start(qh[0:HD, :], d_qs[hc].ap()[off:off + HD, :])
                nc.sync.dma_start(qh[HD:P, :], d_zp.ap())
                return kh, qh

            # spill the first two h'-chunks of Q/K aggregation upfront; the
            # remaining chunks interleave with the head loop two chunks ahead
            for hc0 in (0, 1):
                spill_chunk(q_nat, tt, hc0, d_qs)
                spill_chunk(k_nat, tt, hc0, d_ks)
            pre_kq[0] = _load_head(0)
            pre_kq[1] = _load_head(1)

            # prefetch out-projection weights + folded LN rows during phase 2
            wo_t = big.tile([P, HO, H], BF16, tag="big")
            nc.sync.dma_start(wo_t[:], d_w["o"].ap())
            b1r = s4.tile([1, H], F32, tag="s4")
            nc.sync.dma_start(b1r[:], d_b["o"].ap())
            b1b = cpool.tile([P, H], F32, name="b1b")
            nc.gpsimd.partition_broadcast(b1b[:], b1r[:])
            g1r = s4.tile([1, H], F32, tag="s4")
            nc.sync.dma_start(g1r[:], d_g1n.ap())
            g1nb = cpool.tile([P, H], F32, name="g1nb")
            nc.gpsimd.partition_broadcast(g1nb[:], g1r[:])

            # ---- phase 2: attention. One attention-out tile per h' chunk
            # (separate tensors keep the scheduler from inventing cross-chunk
            # dependencies); LN stats accumulate on the DVE only.
            attn_c = [atp.tile([P, S], BF16, tag="at", name=f"attn{c}")
                      for c in range(HO)]
            acc = accp.tile([P, S], BF16, tag="acc", name="acc")
            acc2 = accp.tile([P, S], BF16, tag="acc", name="acc2")

            rb_c = None
            for h in range(NH):
                hc, off = h // 2, (h % 2) * HD
                # zero-pad the contraction dim to K=128 (rows 64:128 from a
                # DRAM zeros pad) to keep the PE activity profile flat
                if h in pre_kq:
                    kh, qh = pre_kq[h]
                else:
                    kh, qh = _load_head(h)
                if off == 0:
                    if hc + 2 < HO:
                        spill_chunk(q_nat, tt, hc + 2, d_qs)
                        spill_chunk(k_nat, tt, hc + 2, d_ks)
                    rb_c = s4.tile([P, S], F32, tag="s4")
                pv_a = pvps.tile([P, 512], F32, tag="pv", name="pv_a")
                pv_b = pvps.tile([P, 512], F32, tag="pv", name="pv_b")
                # software-pipelined: both query halves of a key chunk land in
                # one two-bank PSUM tile so a single Exp serves the chunk,
                # running AHEAD chunks in front of the PV accumulation
                ets = []
                for kc in range(SO + AHEAD):
                    if kc < SO:
                        sp = scps.tile([P, 1024], F32, tag="sc")
                        nc.tensor.matmul(
                            sp[:, 0:512], kh[0:P, kc * P:(kc + 1) * P],
                            qh[0:P, 0:512],
                            start=True, stop=True, skip_group_check=True)
                        nc.tensor.matmul(
                            sp[:, 512:1024], kh[0:P, kc * P:(kc + 1) * P],
                            qh[0:P, 512:1024],
                            start=True, stop=True, skip_group_check=True)
                        e_t = etp.tile([P, 1024], BF16, tag="et")
                        nc.scalar.activation(
                            e_t[:], sp[:],
                            mybir.ActivationFunctionType.Exp, scale=0.125)
                        ets.append(e_t)
                    if kc >= AHEAD:
                        j = kc - AHEAD
                        nc.tensor.matmul(
                            pv_a[0:HD + 1, :], v_ext[:, j, h, :],
                            ets[j][:, 0:512],
                            start=(j == 0), stop=(j == SO - 1),
                            skip_group_check=True)
                        nc.tensor.matmul(
                            pv_b[0:HD + 1, :], v_ext[:, j, h, :],
                            ets[j][:, 512:1024],
                            start=(j == 0), stop=(j == SO - 1),
                            skip_group_check=True)
                # evict raw out + sums; broadcast sums (no PE dependency).
                # partition_broadcast only writes reliably at partition 0,
                # so odd heads bounce through a temp + DVE copy.
                for q2, pv in ((0, pv_a), (1, pv_b)):
                    qs = slice(q2 * 512, (q2 + 1) * 512)
                    nc.vector.tensor_copy(attn_c[hc][off:off + HD, qs],
                                          pv[0:HD, :])
                    srow = s2.tile([1, 512], F32, tag="s2")
                    nc.vector.tensor_copy(srow[:], pv[HD:HD + 1, :])
                    if off == 0:
                        nc.gpsimd.partition_broadcast(rb_c[0:HD, qs], srow[:])
                    else:
                        tmp = s2.tile([HD, 512], F32, tag="s2")
                        nc.gpsimd.partition_broadcast(tmp[:], srow[:])
                        nc.vector.tensor_copy(rb_c[off:off + HD, qs], tmp[:])
                if off == HD:
                    # chunk hc complete: normalize + LN stats (all DVE)
                    rcp_c = s4.tile([P, S], F32, tag="s4")
                    nc.vector.reciprocal_approx_fast(rcp_c[:], rb_c[:])
                    nc.vector.tensor_tensor(attn_c[hc][:], attn_c[hc][:],
                                            rcp_c[:], mybir.AluOpType.mult)
                    sq_c = sqp.tile([P, S], BF16, tag="sq")
                    nc.vector.tensor_tensor(sq_c[:], attn_c[hc][:],
                                            attn_c[hc][:],
                                            mybir.AluOpType.mult)
                    if hc == 0:
                        nc.vector.tensor_copy(acc[:], attn_c[0][:])
                        nc.vector.tensor_copy(acc2[:], sq_c[:])
                    elif hc < HO - 1:
                        nc.vector.tensor_tensor(acc[:], acc[:], attn_c[hc][:],
                                                mybir.AluOpType.add)
                        nc.vector.tensor_tensor(acc2[:], acc2[:], sq_c[:],
                                                mybir.AluOpType.add)
                    else:
                        sq7 = sq_c  # last chunk's stats go straight to PSUM

            # ---- phase 3: one block of transposed stats matmuls ([s-part,
            # so] layout) + LN scale factors -- all tiny ops
            stat_t = pvps.tile([P, 32], F32, tag="pv", name="stat_t")
            for so in range(SO):
                # chunks 0..6 ride acc/acc2 (hoistable off the tail); chunk 7
                # feeds its own columns directly so only these 16 tiny
                # matmuls trail the last normalize
                nc.tensor.matmul(
                    stat_t[:, so:so + 1], acc[:, so * P:(so + 1) * P],
                    ones_st[:], start=True, stop=True, skip_group_check=True)
                nc.tensor.matmul(
                    stat_t[:, 8 + so:9 + so], acc2[:, so * P:(so + 1) * P],
                    ones_st[:], start=True, stop=True, skip_group_check=True)
                nc.tensor.matmul(
                    stat_t[:, 16 + so:17 + so],
                    attn_c[HO - 1][:, so * P:(so + 1) * P],
                    ones_st[:], start=True, stop=True, skip_group_check=True)
                nc.tensor.matmul(
                    stat_t[:, 24 + so:25 + so], sq7[:, so * P:(so + 1) * P],
                    ones_st[:], start=True, stop=True, skip_group_check=True)
            acc16 = s2.tile([P, 16], F32, tag="s2")
            nc.vector.tensor_copy(acc16[:], stat_t[:, 0:16])
            nc.vector.tensor_tensor(acc16[:], acc16[:], stat_t[:, 16:32],
                                    mybir.AluOpType.add)
            m2 = s2.tile([P, 8], F32, tag="s2")
            nc.scalar.square(m2[:], acc16[:, 0:8])
            var_t = s2.tile([P, 8], F32, tag="s2")
            nc.vector.tensor_tensor(var_t[:], acc16[:, 8:16], m2[:],
                                    mybir.AluOpType.subtract)
            nc.scalar.activation(var_t[:], var_t[:],
                                 mybir.ActivationFunctionType.Sqrt, bias=eps_t[:])
            rstd = s2.tile([P, 8], F32, tag="s2")
            nc.vector.reciprocal_approx_fast(rstd[:], var_t[:])
            rmu = s2.tile([P, 8], F32, tag="s2")
            nc.vector.tensor_tensor(rmu[:], acc16[:, 0:8], rstd[:],
                                    mybir.AluOpType.mult)

            # ---- phase 4: output projection on raw attn with fused LN:
            # y = rstd*(attn^T @ W') - (mu*rstd)*g1 + b1
            for so in range(SO):
                for nh in range(2):
                    t2 = s2.tile([P, 512], F32, tag="s2")
                    nc.vector.scalar_tensor_tensor(
                        t2[:], g1nb[:, nh * 512:(nh + 1) * 512],
                        rmu[:, so:so + 1], b1b[:, nh * 512:(nh + 1) * 512],
                        mybir.AluOpType.mult, mybir.AluOpType.add)
                    ps = mmps.tile([P, 512], F32, tag="mm")
                    for hc in range(HO):
                        nc.tensor.matmul(
                            ps[:], attn_c[hc][:, so * P:(so + 1) * P],
                            wo_t[:, hc, nh * 512:(nh + 1) * 512],
                            start=(hc == 0), stop=(hc == HO - 1))
                    ych = s2.tile([P, 512], F32, tag="s2")
                    nc.vector.scalar_tensor_tensor(
                        ych[:], ps[:], rstd[:, so:so + 1], t2[:],
                        mybir.AluOpType.mult, mybir.AluOpType.add)
                    nc.sync.dma_start(
                        d_y.ap()[:, so, nh * 512:(nh + 1) * 512], ych[:])

    nc.compile()
    return nc


_NC = None


def _get_nc():
    global _NC
    if _NC is None:
        _NC = _build_program()
    return _NC


def _host_inputs(query, key, value, Wq, bq, Wk, bk, Wv, bv, Wo, bo,
                 temporal_weights, ln_gamma, ln_beta):
    T = _temporal_matrix(temporal_weights)
    tt_host = np.ascontiguousarray(  # TT[p, so, s'] = T[s', so*P+p]
        T.T.reshape(SO, P, S).transpose(1, 0, 2)).astype(NPBF16)
    cos, sin = _rope_tables()
    gam = np.asarray(ln_gamma, np.float32)
    bet = np.asarray(ln_beta, np.float32)
    Wo32 = np.asarray(Wo, np.float32)
    Wfold = gam[:, None] * Wo32               # gamma folded into out-proj
    g1n = -(gam @ Wo32).reshape(1, H)         # rank-1 LN correction row
    b1 = (bet @ Wo32 + np.asarray(bo, np.float32)).reshape(1, H)
    common = {
        "w_v": _nat(np.asarray(Wv, np.float32)).astype(NPBF16),
        "w_q": _nat(np.asarray(Wq, np.float32)).astype(NPBF16),
        "w_k": _nat(np.asarray(Wk, np.float32)).astype(NPBF16),
        "w_o": _nat(Wfold).astype(NPBF16),
        "b_v": np.asarray(bv, np.float32).reshape(1, H),
        "b_q": np.asarray(bq, np.float32).reshape(1, H),
        "b_k": np.asarray(bk, np.float32).reshape(1, H),
        "b_o": b1,
        "g1n": g1n,
        "tt": tt_host,
        "zpad": np.zeros((HD, S), NPBF16),
        "cos_t": _nat(cos).astype(NPBF16),
        "sin_t": _nat(sin).astype(NPBF16),
    }
    in_maps = []
    for c in range(N_CORES):
        m = dict(common)
        m["xt_q"] = _xt_chunks(np.asarray(query[c], np.float32)).astype(NPBF16)
        m["xt_k"] = _xt_chunks(np.asarray(key[c], np.float32)).astype(NPBF16)
        m["xt_v"] = _xt_chunks(np.asarray(value[c], np.float32)).astype(NPBF16)
        in_maps.append(m)
    return in_maps


def kernel(query, key, value, Wq, bq, Wk, bk, Wv, bv, Wo, bo,
           temporal_weights, ln_gamma, ln_beta):
    in_maps = _host_inputs(query, key, value, Wq, bq, Wk, bk, Wv, bv, Wo, bo,
                           temporal_weights, ln_gamma, ln_beta)
    nc = _get_nc()
    res = run_bass_kernel_spmd(nc, in_maps, list(range(N_CORES)))
    out = np.empty((B, S, H), np.float32)
    for c in range(N_CORES):
        y = res.results[c]["y"]  # [P, SO, H]
        out[c] = y.transpose(1, 0, 2).reshape(S, H)
    return out


# revision 18
# speedup vs baseline: 1.0167x; 1.0010x over previous
"""HSTU-style attention block (RoPE + multi-scale temporal agg + SDPA + LN + out-proj)
for Trainium2, data-parallel over batch across 8 NeuronCores.

Per-core layout strategy (batch element per core):
  - host pre-transposes X so projections run with activations as lhsT
  - Q/K/V projected into natural [s, h'] layout; RoPE applied in-place in bf16
    (all-bf16 packed operands ride the DVE 2x mode)
  - temporal aggregation applied as a matmul against a host-built [S, S] matrix T
    (softmax(temporal_weights)); band structure (|s'-s| <= 11) trims contraction
    chunks at 256-wide output granularity; Q/K produced transposed, V natural
    with an extra ones column so softmax denominators ride the PV matmul
  - attention computes scores^T per head over the FULL query range: two N=512
    matmuls land in one two-bank [128,1024] PSUM tile so a single scalar-engine
    Exp covers each key chunk (the Exp stream is the phase-2 floor), pipelined
    AHEAD chunks in front of the PV accumulation
  - LayerNorm statistics accumulate on the DVE in bf16 (sum / sum-of-squares
    per chunk); one block of 16 transposed N=1 ones-matmuls at the end of
    phase 2 moves them into the [s-partition] layout, where gamma/beta fold
    into the out-projection weights (host) and the LN apply collapses to a
    per-partition scale at PSUM eviction plus a rank-1 mu*rstd correction
All matmuls run in bfloat16 (fp32 PSUM accumulation).
"""

import numpy as np
import ml_dtypes
import concourse.mybir as mybir
import concourse.tile as tile
from concourse import bacc
from concourse.bass_utils import run_bass_kernel_spmd

B, S, H, NH = 8, 1024, 1024, 16
HD = H // NH  # 64
P = 128
SO = S // P  # 8
HO = H // P  # 8
N_SCALES = 4
LN_EPS = 1e-5
F32 = mybir.dt.float32
BF16 = mybir.dt.bfloat16
NPBF16 = ml_dtypes.bfloat16

N_CORES = 8
AHEAD = 2  # exp pipeline depth (score chunks ahead of PV)


# ---------------------------------------------------------------- host helpers
def _softmax_np(x):
    x = np.asarray(x, np.float64)
    e = np.exp(x - x.max())
    return e / e.sum()


def _temporal_matrix(temporal_weights):
    """[S, S] matrix T with (T @ x) == temporal_agg(x) along the sequence axis."""
    w = _softmax_np(temporal_weights)
    T = np.eye(S, dtype=np.float64) * w[0]
    for scale in range(1, N_SCALES):
        p = max(1, S // (2 ** scale))
        k = S // p
        pool = np.zeros((p, S), dtype=np.float64)
        for j in range(p):
            pool[j, j * k:(j + 1) * k] = 1.0 / k
        coord = (np.arange(S, dtype=np.float64) + 0.5) * (p / S) - 0.5
        coord = np.clip(coord, 0.0, None)
        i0 = np.minimum(np.floor(coord).astype(np.int64), p - 1)
        i1 = np.minimum(i0 + 1, p - 1)
        lam = (coord - i0).astype(np.float32).astype(np.float64)
        interp = np.zeros((S, p), dtype=np.float64)
        interp[np.arange(S), i0] += 1.0 - lam
        interp[np.arange(S), i1] += lam
        T += w[scale] * (interp @ pool)
    return T.astype(np.float32)


def _rope_tables():
    inv_freq = 1.0 / (10000.0 ** (np.arange(0, HD, 2, dtype=np.float64) / HD))
    freqs = np.arange(S, dtype=np.float64)[:, None] * inv_freq[None, :]
    cos = np.repeat(np.cos(freqs), 2, axis=-1).astype(np.float32)  # [S, HD]
    sin = np.repeat(np.sin(freqs), 2, axis=-1).astype(np.float32)
    return cos, sin


def _nat(x):
    """[S, D] -> [P, S//P, D] with x[so*P+p, d] = out[p, so, d]."""
    return np.ascontiguousarray(x.reshape(SO, P, x.shape[-1]).transpose(1, 0, 2))


def _xt_chunks(x):
    """[S, H] -> [P, SO, HO*P] with out[p, so, ho*P + i] = x[so*P + i, ho*P + p]."""
    return np.ascontiguousarray(
        x.reshape(SO, P, HO, P).transpose(3, 0, 2, 1).reshape(P, SO, H))


# ---------------------------------------------------------------- bass program
def _build_program():
    nc = bacc.Bacc("TRN2", target_bir_lowering=False, debug=False)

    d_xt = {a: nc.dram_tensor(f"xt_{a}", [P, SO, H], BF16, kind="ExternalInput")
            for a in ("v", "q", "k")}
    d_w = {a: nc.dram_tensor(f"w_{a}", [P, HO, H], BF16, kind="ExternalInput")
           for a in ("v", "q", "k", "o")}
    d_b = {a: nc.dram_tensor(f"b_{a}", [1, H], F32, kind="ExternalInput")
           for a in ("v", "q", "k", "o")}
    d_g1n = nc.dram_tensor("g1n", [1, H], F32, kind="ExternalInput")
    d_tt = nc.dram_tensor("tt", [P, SO, S], BF16, kind="ExternalInput")
    d_cos = nc.dram_tensor("cos_t", [P, SO, HD], BF16, kind="ExternalInput")
    d_sin = nc.dram_tensor("sin_t", [P, SO, HD], BF16, kind="ExternalInput")
    d_y = nc.dram_tensor("y", [P, SO, H], F32, kind="ExternalOutput")
    d_zp = nc.dram_tensor("zpad", [HD, S], BF16, kind="ExternalInput")
    # per-chunk scratch so a head's reload only waits on its own spill DMA
    d_qs = [nc.dram_tensor(f"q_scr{hc}", [P, S], BF16) for hc in range(HO)]
    d_ks = [nc.dram_tensor(f"k_scr{hc}", [P, S], BF16) for hc in range(HO)]

    with tile.TileContext(nc) as tc:
        with (
            tc.tile_pool(name="const", bufs=1) as cpool,
            tc.tile_pool(name="big", bufs=5) as big,
            tc.tile_pool(name="s4", bufs=6) as s4,
            tc.tile_pool(name="xt", bufs=3) as xtp,
            tc.tile_pool(name="rot", bufs=2) as rotp,
            tc.tile_pool(name="kq", bufs=6) as kqp,
            tc.tile_pool(name="s2", bufs=6) as s2,
            tc.tile_pool(name="et", bufs=4) as etp,
            tc.tile_pool(name="sq", bufs=2) as sqp,
            tc.tile_pool(name="at", bufs=8) as atp,
            tc.tile_pool(name="acc", bufs=2) as accp,
            tc.tile_pool(name="mm_ps", bufs=2, space="PSUM") as mmps,
            tc.tile_pool(name="sc_ps", bufs=2, space="PSUM") as scps,
            tc.tile_pool(name="pv_ps", bufs=2, space="PSUM") as pvps,
        ):
            cos_t = cpool.tile([P, SO, HD], BF16, name="cos_t")
            sin_t = cpool.tile([P, SO, HD], BF16, name="sin_t")
            nc.sync.dma_start(cos_t[:], d_cos.ap())
            nc.sync.dma_start(sin_t[:], d_sin.ap())
            ones = cpool.tile([P, 1], F32, name="ones")
            nc.vector.memset(ones[:], 1.0)
            eps_t = cpool.tile([P, 1], F32, name="eps_t")
            nc.vector.memset(eps_t[:], LN_EPS)
            sqwarm = cpool.tile([P, 1], F32, name="sqwarm")
            nc.scalar.activation(sqwarm[:], eps_t[:],
                                 mybir.ActivationFunctionType.Sqrt)
            # stats rhs: 1/H so the PSUM accumulators hold means directly
            ones_st = cpool.tile([P, 1], BF16, name="ones_st")
            nc.vector.memset(ones_st[:], 1.0 / H)

            def _rope_chunk(a_nat, so):
                """In-place bf16 RoPE on a_nat[:, so, :] (DVE 2x mode)."""
                ch = a_nat[:, so, :]
                ch3 = ch.rearrange("p (nh d) -> p nh d", d=HD)
                ch4 = ch.rearrange("p (nh hf dd) -> p nh hf dd", hf=2, dd=HD // 2)
                rot = rotp.tile([P, H], BF16, tag="rot")
                rot4 = rot[:].rearrange("p (nh hf dd) -> p nh hf dd",
                                        hf=2, dd=HD // 2)
                rot3 = rot[:].rearrange("p (nh d) -> p nh d", d=HD)
                nc.vector.tensor_scalar_mul(rot4[:, :, 0, :], ch4[:, :, 1, :], -1.0)
                nc.vector.tensor_copy(rot4[:, :, 1, :], ch4[:, :, 0, :])
                cb = cos_t[:, so, :][:, None, :].to_broadcast((P, NH, HD))
                sb = sin_t[:, so, :][:, None, :].to_broadcast((P, NH, HD))
                nc.vector.tensor_tensor(ch3[:], ch3[:], cb, mybir.AluOpType.mult)
                nc.vector.tensor_tensor(rot3[:], rot3[:], sb, mybir.AluOpType.mult)
                nc.vector.tensor_tensor(ch[:], ch[:], rot[:], mybir.AluOpType.add)

            def project(a, do_rope=False):
                """A_nat [P, SO, H] (bf16) = X @ W_a + b_a, optional fused RoPE."""
                w_t = big.tile([P, HO, H], BF16, tag="big")
                for ko in range(HO):
                    nc.sync.dma_start(w_t[:, ko, :], d_w[a].ap()[:, ko, :])
                brow = s4.tile([1, H], F32, tag="s4")
                nc.sync.dma_start(brow[:], d_b[a].ap())
                bb = s4.tile([P, H], F32, tag="s4")
                nc.gpsimd.partition_broadcast(bb[:], brow[:])
                a_nat = big.tile([P, SO, H], BF16, tag="big")
                for so in range(SO):
                    xt_c = xtp.tile([P, HO, P], BF16, tag="xt")
                    nc.sync.dma_start(xt_c[:], d_xt[a].ap()[:, so, :])
                    for nh in range(2):
                        ps = mmps.tile([P, 512], F32, tag="mm")
                        for ko in range(HO):
                            nc.tensor.matmul(
                                ps[:], xt_c[:, ko, :],
                                w_t[:, ko, nh * 512:(nh + 1) * 512],
                                start=(ko == 0), stop=(ko == HO - 1))
                        nc.vector.tensor_tensor(
                            a_nat[:, so, nh * 512:(nh + 1) * 512], ps[:],
                            bb[:, nh * 512:(nh + 1) * 512], mybir.AluOpType.add)
                    if do_rope:
                        _rope_chunk(a_nat, so)
                return a_nat

            BAND = 12  # T[s', s] == 0 for |s' - s| > 11 (structural)

            def _band_sos(o0, o1):
                """so chunks whose s-range intersects [o0-BAND, o1+BAND)."""
                return [so for so in range(SO)
                        if so * P + P > o0 - BAND and so * P < o1 + BAND]

            def spill_chunk(a_nat, tt, hc, d_scr):
                """One h'-chunk of (T @ A).T spilled to DRAM scratch. Runs
                interleaved with attention, so the eviction rides the DVE
                (the scalar engine is saturated by the Exp stream there)."""
                for sh in range(2):
                    ps = mmps.tile([P, 512], F32, tag="mm")
                    for q in range(2):
                        o0 = sh * 512 + q * 256
                        sos = _band_sos(o0, o0 + 256)
                        for so in sos:
                            nc.tensor.matmul(
                                ps[:, q * 256:(q + 1) * 256],
                                a_nat[:, so, hc * P:(hc + 1) * P],
                                tt[:, so, o0:o0 + 256],
                                start=(so == sos[0]), stop=(so == sos[-1]),
                                skip_group_check=True)
                    ev = s2.tile([P, 512], BF16, tag="s2")
                    nc.vector.tensor_copy(ev[:], ps[:])
                    nc.sync.dma_start(
                        d_scr[hc].ap()[:, sh * 512:(sh + 1) * 512], ev[:])

            def t_agg_v(v_nat, tt):
                """V_ext [P, SO, NH, HD+1] (bf16) = T @ V with ones column."""
                v_ext = big.tile([P, SO, NH, HD + 1], BF16, tag="big")
                nc.vector.tensor_copy(
                    v_ext[:, :, :, HD:HD + 1],
                    ones[:, None, None, :].to_broadcast((P, SO, NH, 1)))
                for sc in range(SO):
                    sos = _band_sos(sc * P, (sc + 1) * P)
                    for dh in range(2):
                        ps = mmps.tile([P, 512], F32, tag="mm")
                        for so in sos:
                            nc.tensor.matmul(
                                ps[:], tt[:, so, sc * P:(sc + 1) * P],
                                v_nat[:, so, dh * 512:(dh + 1) * 512],
                                start=(so == sos[0]), stop=(so == sos[-1]))
                        pvw = ps[:].rearrange("p (nh d) -> p nh d", d=HD)
                        nc.scalar.copy(
                            v_ext[:, sc, dh * 8:(dh + 1) * 8, 0:HD], pvw)
                return v_ext

            # ---- phase 1: V, Q, K  (projection + RoPE + temporal aggregation)
            v_nat = project("v")
            tt = big.tile([P, SO, S], BF16, tag="big")
            nc.sync.dma_start(tt[:], d_tt.ap())
            v_ext = t_agg_v(v_nat, tt)

            q_nat = project("q", do_rope=True)
            k_nat = project("k", do_rope=True)

            pre_kq = {}

            def _load_head(h):
                off = (h % 2) * HD
                hc = h // 2
                kh = kqp.tile([P, S], BF16, tag="kq", name=f"kh{h}")
                nc.sync.dma_start(kh[0:HD, :], d_ks[hc].ap()[off:off + HD, :])
                nc.sync.dma_start(kh[HD:P, :], d_zp.ap())
                qh = kqp.tile([P, S], BF16, tag="kq", name=f"qh{h}")
                nc.sync.dma_start(qh[0:HD, :], d_qs[hc].ap()[off:off + HD, :])
                nc.sync.dma_start(qh[HD:P, :], d_zp.ap())
                return kh, qh

            # spill the first two h'-chunks of Q/K aggregation upfront; the
            # remaining chunks interleave with the head loop two chunks ahead
            for hc0 in (0, 1):
                spill_chunk(q_nat, tt, hc0, d_qs)
                spill_chunk(k_nat, tt, hc0, d_ks)
            pre_kq[0] = _load_head(0)
            pre_kq[1] = _load_head(1)

            # prefetch out-projection weights + folded LN rows during phase 2
            wo_t = big.tile([P, HO, H], BF16, tag="big")
            nc.sync.dma_start(wo_t[:], d_w["o"].ap())
            b1r = s4.tile([1, H], F32, tag="s4")
            nc.sync.dma_start(b1r[:], d_b["o"].ap())
            b1b = cpool.tile([P, H], F32, name="b1b")
            nc.gpsimd.partition_broadcast(b1b[:], b1r[:])
            g1r = s4.tile([1, H], F32, tag="s4")
            nc.sync.dma_start(g1r[:], d_g1n.ap())
            g1nb = cpool.tile([P, H], F32, name="g1nb")
            nc.gpsimd.partition_broadcast(g1nb[:], g1r[:])

            # ---- phase 2: attention. One attention-out tile per h' chunk
            # (separate tensors keep the scheduler from inventing cross-chunk
            # dependencies); LN stats accumulate on the DVE only.
            attn_c = [atp.tile([P, S], BF16, tag="at", name=f"attn{c}")
                      for c in range(HO)]
            acc = accp.tile([P, S], BF16, tag="acc", name="acc")
            acc2 = accp.tile([P, S], BF16, tag="acc", name="acc2")

            rb_c = None
            for h in range(NH):
                hc, off = h // 2, (h % 2) * HD
                # zero-pad the contraction dim to K=128 (rows 64:128 from a
                # DRAM zeros pad) to keep the PE activity profile flat
                if h in pre_kq:
                    kh, qh = pre_kq[h]
                else:
                    kh, qh = _load_head(h)
                if off == 0:
                    if hc + 2 < HO:
                        spill_chunk(q_nat, tt, hc + 2, d_qs)
                        spill_chunk(k_nat, tt, hc + 2, d_ks)
                    rb_c = s4.tile([P, S], F32, tag="s4")
                    rcp_c = s4.tile([P, S], F32, tag="s4")
                else:
                    # even head's rows normalize while the odd head computes,
                    # halving the serial chain at every chunk boundary. The
                    # custom reciprocal op must start at partition 0 (its
                    # seed constants are partition-aligned), so it runs on
                    # the full tile; the top half is recomputed at the
                    # boundary once the odd head's denominators land.
                    nc.vector.reciprocal_approx_fast(rcp_c[:], rb_c[:])
                    nc.vector.tensor_tensor(attn_c[hc][0:HD, :],
                                            attn_c[hc][0:HD, :],
                                            rcp_c[0:HD, :],
                                            mybir.AluOpType.mult)
                pv_a = pvps.tile([P, 512], F32, tag="pv", name="pv_a")
                pv_b = pvps.tile([P, 512], F32, tag="pv", name="pv_b")
                # software-pipelined: both query halves of a key chunk land in
                # one two-bank PSUM tile so a single Exp serves the chunk,
                # running AHEAD chunks in front of the PV accumulation
                ets = []
                for kc in range(SO + AHEAD):
                    if kc < SO:
                        sp = scps.tile([P, 1024], F32, tag="sc")
                        nc.tensor.matmul(
                            sp[:, 0:512], kh[0:P, kc * P:(kc + 1) * P],
                            qh[0:P, 0:512],
                            start=True, stop=True, skip_group_check=True)
                        nc.tensor.matmul(
                            sp[:, 512:1024], kh[0:P, kc * P:(kc + 1) * P],
                            qh[0:P, 512:1024],
                            start=True, stop=True, skip_group_check=True)
                        e_t = etp.tile([P, 1024], BF16, tag="et")
                        nc.scalar.activation(
                            e_t[:], sp[:],
                            mybir.ActivationFunctionType.Exp, scale=0.125)
                        ets.append(e_t)
                    if kc >= AHEAD:
                        j = kc - AHEAD
                        nc.tensor.matmul(
                            pv_a[0:HD + 1, :], v_ext[:, j, h, :],
                            ets[j][:, 0:512],
                            start=(j == 0), stop=(j == SO - 1),
                            skip_group_check=True)
                        nc.tensor.matmul(
                            pv_b[0:HD + 1, :], v_ext[:, j, h, :],
                            ets[j][:, 512:1024],
                            start=(j == 0), stop=(j == SO - 1),
                            skip_group_check=True)
                # evict raw out + sums; broadcast sums (no PE dependency).
                # partition_broadcast only writes reliably at partition 0,
                # so odd heads bounce through a temp + DVE copy.
                for q2, pv in ((0, pv_a), (1, pv_b)):
                    qs = slice(q2 * 512, (q2 + 1) * 512)
                    nc.vector.tensor_copy(attn_c[hc][off:off + HD, qs],
                                          pv[0:HD, :])
                    srow = s2.tile([1, 512], F32, tag="s2")
                    nc.vector.tensor_copy(srow[:], pv[HD:HD + 1, :])
                    if off == 0:
                        nc.gpsimd.partition_broadcast(rb_c[0:HD, qs], srow[:])
                    else:
                        tmp = s2.tile([HD, 512], F32, tag="s2")
                        nc.gpsimd.partition_broadcast(tmp[:], srow[:])
                        nc.vector.tensor_copy(rb_c[off:off + HD, qs], tmp[:])
                if off == HD:
                    # chunk hc complete: odd head's rows + LN stats (all DVE)
                    rcp2 = s4.tile([P, S], F32, tag="s4")
                    nc.vector.reciprocal_approx_fast(rcp2[:], rb_c[:])
                    nc.vector.tensor_tensor(attn_c[hc][HD:P, :],
                                            attn_c[hc][HD:P, :],
                                            rcp2[HD:P, :],
                                            mybir.AluOpType.mult)
                    sq_c = sqp.tile([P, S], BF16, tag="sq")
                    nc.vector.tensor_tensor(sq_c[:], attn_c[hc][:],
                                            attn_c[hc][:],
                                            mybir.AluOpType.mult)
                    if hc == 0:
                        nc.vector.tensor_copy(acc[:], attn_c[0][:])
                        nc.vector.tensor_copy(acc2[:], sq_c[:])
                    elif hc < HO - 1:
                        nc.vector.tensor_tensor(acc[:], acc[:], attn_c[hc][:],
                                                mybir.AluOpType.add)
                        nc.vector.tensor_tensor(acc2[:], acc2[:], sq_c[:],
                                                mybir.AluOpType.add)
                    else:
                        sq7 = sq_c  # last chunk's stats go straight to PSUM

            # ---- phase 3: one block of transposed stats matmuls ([s-part,
            # so] layout) + LN scale factors -- all tiny ops
            stat_t = pvps.tile([P, 32], F32, tag="pv", name="stat_t")
            for so in range(SO):
                # chunks 0..6 ride acc/acc2 (hoistable off the tail); chunk 7
                # feeds its own columns directly so only these 16 tiny
                # matmuls trail the last normalize
                nc.tensor.matmul(
                    stat_t[:, so:so + 1], acc[:, so * P:(so + 1) * P],
                    ones_st[:], start=True, stop=True, skip_group_check=True)
                nc.tensor.matmul(
                    stat_t[:, 8 + so:9 + so], acc2[:, so * P:(so + 1) * P],
                    ones_st[:], start=True, stop=True, skip_group_check=True)
                nc.tensor.matmul(
                    stat_t[:, 16 + so:17 + so],
                    attn_c[HO - 1][:, so * P:(so + 1) * P],
                    ones_st[:], start=True, stop=True, skip_group_check=True)
                nc.tensor.matmul(
                    stat_t[:, 24 + so:25 + so], sq7[:, so * P:(so + 1) * P],
                    ones_st[:], start=True, stop=True, skip_group_check=True)
            acc16 = s2.tile([P, 16], F32, tag="s2")
            nc.vector.tensor_copy(acc16[:], stat_t[:, 0:16])
            nc.vector.tensor_tensor(acc16[:], acc16[:], stat_t[:, 16:32],
                                    mybir.AluOpType.add)
            m2 = s2.tile([P, 8], F32, tag="s2")
            nc.scalar.square(m2[:], acc16[:, 0:8])
            var_t = s2.tile([P, 8], F32, tag="s2")
            nc.vector.tensor_tensor(var_t[:], acc16[:, 8:16], m2[:],
                                    mybir.AluOpType.subtract)
            nc.scalar.activation(var_t[:], var_t[:],
                                 mybir.ActivationFunctionType.Sqrt, bias=eps_t[:])
            rstd = s2.tile([P, 8], F32, tag="s2")
            nc.vector.reciprocal_approx_fast(rstd[:], var_t[:])
            rmu = s2.tile([P, 8], F32, tag="s2")
            nc.vector.tensor_tensor(rmu[:], acc16[:, 0:8], rstd[:],
                                    mybir.AluOpType.mult)

            # ---- phase 4: output projection on raw attn with fused LN:
            # y = rstd*(attn^T @ W') - (mu*rstd)*g1 + b1
            for so in range(SO):
                for nh in range(2):
                    ps = mmps.tile([P, 512], F32, tag="mm")
                    for hc in range(HO):
                        nc.tensor.matmul(
                            ps[:], attn_c[hc][:, so * P:(so + 1) * P],
                            wo_t[:, hc, nh * 512:(nh + 1) * 512],
                            start=(hc == 0), stop=(hc == HO - 1))
                    t2 = s2.tile([P, 512], F32, tag="s2")
                    nc.vector.scalar_tensor_tensor(
                        t2[:], g1nb[:, nh * 512:(nh + 1) * 512],
                        rmu[:, so:so + 1], b1b[:, nh * 512:(nh + 1) * 512],
                        mybir.AluOpType.mult, mybir.AluOpType.add)
                    ych = s2.tile([P, 512], F32, tag="s2")
                    nc.vector.scalar_tensor_tensor(
                        ych[:], ps[:], rstd[:, so:so + 1], t2[:],
                        mybir.AluOpType.mult, mybir.AluOpType.add)
                    nc.sync.dma_start(
                        d_y.ap()[:, so, nh * 512:(nh + 1) * 512], ych[:])

    nc.compile()
    return nc


_NC = None


def _get_nc():
    global _NC
    if _NC is None:
        _NC = _build_program()
    return _NC


def _host_inputs(query, key, value, Wq, bq, Wk, bk, Wv, bv, Wo, bo,
                 temporal_weights, ln_gamma, ln_beta):
    T = _temporal_matrix(temporal_weights)
    tt_host = np.ascontiguousarray(  # TT[p, so, s'] = T[s', so*P+p]
        T.T.reshape(SO, P, S).transpose(1, 0, 2)).astype(NPBF16)
    cos, sin = _rope_tables()
    gam = np.asarray(ln_gamma, np.float32)
    bet = np.asarray(ln_beta, np.float32)
    Wo32 = np.asarray(Wo, np.float32)
    Wfold = gam[:, None] * Wo32               # gamma folded into out-proj
    g1n = -(gam @ Wo32).reshape(1, H)         # rank-1 LN correction row
    b1 = (bet @ Wo32 + np.asarray(bo, np.float32)).reshape(1, H)
    common = {
        "w_v": _nat(np.asarray(Wv, np.float32)).astype(NPBF16),
        "w_q": _nat(np.asarray(Wq, np.float32)).astype(NPBF16),
        "w_k": _nat(np.asarray(Wk, np.float32)).astype(NPBF16),
        "w_o": _nat(Wfold).astype(NPBF16),
        "b_v": np.asarray(bv, np.float32).reshape(1, H),
        "b_q": np.asarray(bq, np.float32).reshape(1, H),
        "b_k": np.asarray(bk, np.float32).reshape(1, H),
        "b_o": b1,
        "g1n": g1n,
        "tt": tt_host,
        "zpad": np.zeros((HD, S), NPBF16),
        "cos_t": _nat(cos).astype(NPBF16),
        "sin_t": _nat(sin).astype(NPBF16),
    }
    in_maps = []
    for c in range(N_CORES):
        m = dict(common)
        m["xt_q"] = _xt_chunks(np.asarray(query[c], np.float32)).astype(NPBF16)
        m["xt_k"] = _xt_chunks(np.asarray(key[c], np.float32)).astype(NPBF16)
        m["xt_v"] = _xt_chunks(np.asarray(value[c], np.float32)).astype(NPBF16)
        in_maps.append(m)
    return in_maps


def kernel(query, key, value, Wq, bq, Wk, bk, Wv, bv, Wo, bo,
           temporal_weights, ln_gamma, ln_beta):
    in_maps = _host_inputs(query, key, value, Wq, bq, Wk, bk, Wv, bv, Wo, bo,
                           temporal_weights, ln_gamma, ln_beta)
    nc = _get_nc()
    res = run_bass_kernel_spmd(nc, in_maps, list(range(N_CORES)))
    out = np.empty((B, S, H), np.float32)
    for c in range(N_CORES):
        y = res.results[c]["y"]  # [P, SO, H]
        out[c] = y.transpose(1, 0, 2).reshape(S, H)
    return out


# revision 19
# speedup vs baseline: 1.0210x; 1.0042x over previous
"""HSTU-style attention block (RoPE + multi-scale temporal agg + SDPA + LN + out-proj)
for Trainium2, data-parallel over batch across 8 NeuronCores.

Per-core layout strategy (batch element per core):
  - host pre-transposes X so projections run with activations as lhsT
  - Q/K/V projected into natural [s, h'] layout; RoPE applied in-place in bf16
    (all-bf16 packed operands ride the DVE 2x mode)
  - temporal aggregation applied as a matmul against a host-built [S, S] matrix T
    (softmax(temporal_weights)); band structure (|s'-s| <= 11) trims contraction
    chunks at 256-wide output granularity; Q/K produced transposed, V natural
    with an extra ones column so softmax denominators ride the PV matmul
  - attention computes scores^T per head over the FULL query range: two N=512
    matmuls land in one two-bank [128,1024] PSUM tile so a single scalar-engine
    Exp covers each key chunk (the Exp stream is the phase-2 floor), pipelined
    AHEAD chunks in front of the PV accumulation
  - LayerNorm statistics accumulate on the DVE in bf16 (sum / sum-of-squares
    per chunk); one block of 16 transposed N=1 ones-matmuls at the end of
    phase 2 moves them into the [s-partition] layout, where gamma/beta fold
    into the out-projection weights (host) and the LN apply collapses to a
    per-partition scale at PSUM eviction plus a rank-1 mu*rstd correction
All matmuls run in bfloat16 (fp32 PSUM accumulation).
"""

import numpy as np
import ml_dtypes
import concourse.mybir as mybir
import concourse.tile as tile
from concourse import bacc
from concourse.bass_utils import run_bass_kernel_spmd

B, S, H, NH = 8, 1024, 1024, 16
HD = H // NH  # 64
P = 128
SO = S // P  # 8
HO = H // P  # 8
N_SCALES = 4
LN_EPS = 1e-5
F32 = mybir.dt.float32
BF16 = mybir.dt.bfloat16
NPBF16 = ml_dtypes.bfloat16

N_CORES = 8
AHEAD = 2  # exp pipeline depth (score chunks ahead of PV)


# ---------------------------------------------------------------- host helpers
def _softmax_np(x):
    x = np.asarray(x, np.float64)
    e = np.exp(x - x.max())
    return e / e.sum()


def _temporal_matrix(temporal_weights):
    """[S, S] matrix T with (T @ x) == temporal_agg(x) along the sequence axis."""
    w = _softmax_np(temporal_weights)
    T = np.eye(S, dtype=np.float64) * w[0]
    for scale in range(1, N_SCALES):
        p = max(1, S // (2 ** scale))
        k = S // p
        pool = np.zeros((p, S), dtype=np.float64)
        for j in range(p):
            pool[j, j * k:(j + 1) * k] = 1.0 / k
        coord = (np.arange(S, dtype=np.float64) + 0.5) * (p / S) - 0.5
        coord = np.clip(coord, 0.0, None)
        i0 = np.minimum(np.floor(coord).astype(np.int64), p - 1)
        i1 = np.minimum(i0 + 1, p - 1)
        lam = (coord - i0).astype(np.float32).astype(np.float64)
        interp = np.zeros((S, p), dtype=np.float64)
        interp[np.arange(S), i0] += 1.0 - lam
        interp[np.arange(S), i1] += lam
        T += w[scale] * (interp @ pool)
    return T.astype(np.float32)


def _rope_tables():
    inv_freq = 1.0 / (10000.0 ** (np.arange(0, HD, 2, dtype=np.float64) / HD))
    freqs = np.arange(S, dtype=np.float64)[:, None] * inv_freq[None, :]
    cos = np.repeat(np.cos(freqs), 2, axis=-1).astype(np.float32)  # [S, HD]
    sin = np.repeat(np.sin(freqs), 2, axis=-1).astype(np.float32)
    return cos, sin


def _nat(x):
    """[S, D] -> [P, S//P, D] with x[so*P+p, d] = out[p, so, d]."""
    return np.ascontiguousarray(x.reshape(SO, P, x.shape[-1]).transpose(1, 0, 2))


def _xt_chunks(x):
    """[S, H] -> [P, SO, HO*P] with out[p, so, ho*P + i] = x[so*P + i, ho*P + p]."""
    return np.ascontiguousarray(
        x.reshape(SO, P, HO, P).transpose(3, 0, 2, 1).reshape(P, SO, H))


# ---------------------------------------------------------------- bass program
def _build_program():
    nc = bacc.Bacc("TRN2", target_bir_lowering=False, debug=False)

    d_xt = {a: nc.dram_tensor(f"xt_{a}", [P, SO, H], BF16, kind="ExternalInput")
            for a in ("v", "q", "k")}
    d_w = {a: nc.dram_tensor(f"w_{a}", [P, HO, H], BF16, kind="ExternalInput")
           for a in ("v", "q", "k", "o")}
    d_b = {a: nc.dram_tensor(f"b_{a}", [1, H], F32, kind="ExternalInput")
           for a in ("v", "q", "k", "o")}
    d_g1n = nc.dram_tensor("g1n", [1, H], F32, kind="ExternalInput")
    d_tt = nc.dram_tensor("tt", [P, SO, S], BF16, kind="ExternalInput")
    d_cos = nc.dram_tensor("cos_t", [P, SO, HD], BF16, kind="ExternalInput")
    d_sin = nc.dram_tensor("sin_t", [P, SO, HD], BF16, kind="ExternalInput")
    d_y = nc.dram_tensor("y", [P, SO, H], F32, kind="ExternalOutput")
    d_zp = nc.dram_tensor("zpad", [HD, S], BF16, kind="ExternalInput")
    # per-chunk scratch so a head's reload only waits on its own spill DMA
    d_qs = [nc.dram_tensor(f"q_scr{hc}", [P, S], BF16) for hc in range(HO)]
    d_ks = [nc.dram_tensor(f"k_scr{hc}", [P, S], BF16) for hc in range(HO)]

    with tile.TileContext(nc) as tc:
        with (
            tc.tile_pool(name="const", bufs=1) as cpool,
            tc.tile_pool(name="big", bufs=5) as big,
            tc.tile_pool(name="s4", bufs=6) as s4,
            tc.tile_pool(name="xt", bufs=3) as xtp,
            tc.tile_pool(name="rot", bufs=2) as rotp,
            tc.tile_pool(name="kq", bufs=6) as kqp,
            tc.tile_pool(name="s2", bufs=6) as s2,
            tc.tile_pool(name="et", bufs=4) as etp,
            tc.tile_pool(name="sq", bufs=2) as sqp,
            tc.tile_pool(name="at", bufs=8) as atp,
            tc.tile_pool(name="acc", bufs=2) as accp,
            tc.tile_pool(name="mm_ps", bufs=2, space="PSUM") as mmps,
            tc.tile_pool(name="sc_ps", bufs=2, space="PSUM") as scps,
            tc.tile_pool(name="pv_ps", bufs=2, space="PSUM") as pvps,
        ):
            cos_t = cpool.tile([P, SO, HD], BF16, name="cos_t")
            sin_t = cpool.tile([P, SO, HD], BF16, name="sin_t")
            nc.sync.dma_start(cos_t[:], d_cos.ap())
            nc.sync.dma_start(sin_t[:], d_sin.ap())
            ones = cpool.tile([P, 1], F32, name="ones")
            nc.vector.memset(ones[:], 1.0)
            eps_t = cpool.tile([P, 1], F32, name="eps_t")
            nc.vector.memset(eps_t[:], LN_EPS)
            sqwarm = cpool.tile([P, 1], F32, name="sqwarm")
            nc.scalar.activation(sqwarm[:], eps_t[:],
                                 mybir.ActivationFunctionType.Sqrt)
            # stats rhs: 1/H so the PSUM accumulators hold means directly
            ones_st = cpool.tile([P, 1], BF16, name="ones_st")
            nc.vector.memset(ones_st[:], 1.0 / H)

            def _rope_chunk(a_nat, so):
                """In-place bf16 RoPE on a_nat[:, so, :] (DVE 2x mode)."""
                ch = a_nat[:, so, :]
                ch3 = ch.rearrange("p (nh d) -> p nh d", d=HD)
                ch4 = ch.rearrange("p (nh hf dd) -> p nh hf dd", hf=2, dd=HD // 2)
                rot = rotp.tile([P, H], BF16, tag="rot")
                rot4 = rot[:].rearrange("p (nh hf dd) -> p nh hf dd",
                                        hf=2, dd=HD // 2)
                rot3 = rot[:].rearrange("p (nh d) -> p nh d", d=HD)
                nc.vector.tensor_scalar_mul(rot4[:, :, 0, :], ch4[:, :, 1, :], -1.0)
                nc.vector.tensor_copy(rot4[:, :, 1, :], ch4[:, :, 0, :])
                cb = cos_t[:, so, :][:, None, :].to_broadcast((P, NH, HD))
                sb = sin_t[:, so, :][:, None, :].to_broadcast((P, NH, HD))
                nc.vector.tensor_tensor(ch3[:], ch3[:], cb, mybir.AluOpType.mult)
                nc.vector.tensor_tensor(rot3[:], rot3[:], sb, mybir.AluOpType.mult)
                nc.vector.tensor_tensor(ch[:], ch[:], rot[:], mybir.AluOpType.add)

            def project(a, do_rope=False):
                """A_nat [P, SO, H] (bf16) = X @ W_a + b_a, optional fused RoPE."""
                w_t = big.tile([P, HO, H], BF16, tag="big")
                for ko in range(HO):
                    nc.sync.dma_start(w_t[:, ko, :], d_w[a].ap()[:, ko, :])
                brow = s4.tile([1, H], F32, tag="s4")
                nc.sync.dma_start(brow[:], d_b[a].ap())
                bb = s4.tile([P, H], F32, tag="s4")
                nc.gpsimd.partition_broadcast(bb[:], brow[:])
                a_nat = big.tile([P, SO, H], BF16, tag="big")
                for so in range(SO):
                    xt_c = xtp.tile([P, HO, P], BF16, tag="xt")
                    nc.sync.dma_start(xt_c[:], d_xt[a].ap()[:, so, :])
                    for nh in range(2):
                        ps = mmps.tile([P, 512], F32, tag="mm")
                        for ko in range(HO):
                            nc.tensor.matmul(
                                ps[:], xt_c[:, ko, :],
                                w_t[:, ko, nh * 512:(nh + 1) * 512],
                                start=(ko == 0), stop=(ko == HO - 1))
                        nc.vector.tensor_tensor(
                            a_nat[:, so, nh * 512:(nh + 1) * 512], ps[:],
                            bb[:, nh * 512:(nh + 1) * 512], mybir.AluOpType.add)
                    if do_rope:
                        _rope_chunk(a_nat, so)
                return a_nat

            BAND = 12  # T[s', s] == 0 for |s' - s| > 11 (structural)

            def _band_sos(o0, o1):
                """so chunks whose s-range intersects [o0-BAND, o1+BAND)."""
                return [so for so in range(SO)
                        if so * P + P > o0 - BAND and so * P < o1 + BAND]

            def spill_chunk(a_nat, tt, hc, d_scr):
                """One h'-chunk of (T @ A).T spilled to DRAM scratch. Runs
                interleaved with attention, so the eviction rides the DVE
                (the scalar engine is saturated by the Exp stream there)."""
                for sh in range(2):
                    ps = mmps.tile([P, 512], F32, tag="mm")
                    for q in range(2):
                        o0 = sh * 512 + q * 256
                        sos = _band_sos(o0, o0 + 256)
                        for so in sos:
                            nc.tensor.matmul(
                                ps[:, q * 256:(q + 1) * 256],
                                a_nat[:, so, hc * P:(hc + 1) * P],
                                tt[:, so, o0:o0 + 256],
                                start=(so == sos[0]), stop=(so == sos[-1]),
                                skip_group_check=True)
                    ev = s2.tile([P, 512], BF16, tag="s2")
                    nc.vector.tensor_copy(ev[:], ps[:])
                    nc.sync.dma_start(
                        d_scr[hc].ap()[:, sh * 512:(sh + 1) * 512], ev[:])

            def t_agg_v(v_nat, tt):
                """V_ext [P, SO, NH, HD+1] (bf16) = T @ V with ones column."""
                v_ext = big.tile([P, SO, NH, HD + 1], BF16, tag="big")
                nc.vector.tensor_copy(
                    v_ext[:, :, :, HD:HD + 1],
                    ones[:, None, None, :].to_broadcast((P, SO, NH, 1)))
                for sc in range(SO):
                    sos = _band_sos(sc * P, (sc + 1) * P)
                    for dh in range(2):
                        ps = mmps.tile([P, 512], F32, tag="mm")
                        for so in sos:
                            nc.tensor.matmul(
                                ps[:], tt[:, so, sc * P:(sc + 1) * P],
                                v_nat[:, so, dh * 512:(dh + 1) * 512],
                                start=(so == sos[0]), stop=(so == sos[-1]))
                        pvw = ps[:].rearrange("p (nh d) -> p nh d", d=HD)
                        nc.scalar.copy(
                            v_ext[:, sc, dh * 8:(dh + 1) * 8, 0:HD], pvw)
                return v_ext

            # ---- phase 1: V, Q, K  (projection + RoPE + temporal aggregation)
            v_nat = project("v")
            tt = big.tile([P, SO, S], BF16, tag="big")
            nc.sync.dma_start(tt[:], d_tt.ap())
            v_ext = t_agg_v(v_nat, tt)

            q_nat = project("q", do_rope=True)
            k_nat = project("k", do_rope=True)

            pre_kq = {}
            # six persistent reload buffers: rows 64:128 are zeroed once and
            # never rewritten (reload DMAs only touch rows 0:64), so the
            # K=128 zero-padding costs no per-head DMA traffic
            kq_bufs = [kqp.tile([P, S], BF16, tag="kq", name=f"kqb{i}")
                       for i in range(6)]
            for b in kq_bufs:
                nc.vector.memset(b[HD:P, :], 0.0)

            def _load_head(h):
                off = (h % 2) * HD
                hc = h // 2
                kh = kq_bufs[(2 * h) % 6]
                nc.sync.dma_start(kh[0:HD, :], d_ks[hc].ap()[off:off + HD, :])
                qh = kq_bufs[(2 * h + 1) % 6]
                nc.sync.dma_start(qh[0:HD, :], d_qs[hc].ap()[off:off + HD, :])
                return kh, qh

            # spill the first two h'-chunks of Q/K aggregation upfront; the
            # remaining chunks interleave with the head loop two chunks ahead
            for hc0 in (0, 1):
                spill_chunk(q_nat, tt, hc0, d_qs)
                spill_chunk(k_nat, tt, hc0, d_ks)
            pre_kq[0] = _load_head(0)
            pre_kq[1] = _load_head(1)

            # prefetch out-projection weights + folded LN rows during phase 2
            wo_t = big.tile([P, HO, H], BF16, tag="big")
            nc.sync.dma_start(wo_t[:], d_w["o"].ap())
            b1r = s4.tile([1, H], F32, tag="s4")
            nc.sync.dma_start(b1r[:], d_b["o"].ap())
            b1b = cpool.tile([P, H], F32, name="b1b")
            nc.gpsimd.partition_broadcast(b1b[:], b1r[:])
            g1r = s4.tile([1, H], F32, tag="s4")
            nc.sync.dma_start(g1r[:], d_g1n.ap())
            g1nb = cpool.tile([P, H], F32, name="g1nb")
            nc.gpsimd.partition_broadcast(g1nb[:], g1r[:])

            # ---- phase 2: attention. One attention-out tile per h' chunk
            # (separate tensors keep the scheduler from inventing cross-chunk
            # dependencies); LN stats accumulate on the DVE only.
            attn_c = [atp.tile([P, S], BF16, tag="at", name=f"attn{c}")
                      for c in range(HO)]
            acc = accp.tile([P, S], BF16, tag="acc", name="acc")
            acc2 = accp.tile([P, S], BF16, tag="acc", name="acc2")

            rb_c = None
            for h in range(NH):
                hc, off = h // 2, (h % 2) * HD
                # zero-pad the contraction dim to K=128 (rows 64:128 from a
                # DRAM zeros pad) to keep the PE activity profile flat
                if h in pre_kq:
                    kh, qh = pre_kq[h]
                else:
                    kh, qh = _load_head(h)
                if off == 0:
                    if hc + 2 < HO:
                        spill_chunk(q_nat, tt, hc + 2, d_qs)
                        spill_chunk(k_nat, tt, hc + 2, d_ks)
                    rb_c = s4.tile([P, S], F32, tag="s4")
                    rcp_c = s4.tile([P, S], F32, tag="s4")
                else:
                    # even head's rows normalize while the odd head computes,
                    # halving the serial chain at every chunk boundary. The
                    # custom reciprocal op must start at partition 0 (its
                    # seed constants are partition-aligned), so it runs on
                    # the full tile; the top half is recomputed at the
                    # boundary once the odd head's denominators land.
                    nc.vector.reciprocal_approx_fast(rcp_c[:], rb_c[:])
                    nc.vector.tensor_tensor(attn_c[hc][0:HD, :],
                                            attn_c[hc][0:HD, :],
                                            rcp_c[0:HD, :],
                                            mybir.AluOpType.mult)
                pv_a = pvps.tile([P, 512], F32, tag="pv", name="pv_a")
                pv_b = pvps.tile([P, 512], F32, tag="pv", name="pv_b")
                # software-pipelined: both query halves of a key chunk land in
                # one two-bank PSUM tile so a single Exp serves the chunk,
                # running AHEAD chunks in front of the PV accumulation
                ets = []
                for kc in range(SO + AHEAD):
                    if kc < SO:
                        sp = scps.tile([P, 1024], F32, tag="sc")
                        nc.tensor.matmul(
                            sp[:, 0:512], kh[0:P, kc * P:(kc + 1) * P],
                            qh[0:P, 0:512],
                            start=True, stop=True, skip_group_check=True)
                        nc.tensor.matmul(
                            sp[:, 512:1024], kh[0:P, kc * P:(kc + 1) * P],
                            qh[0:P, 512:1024],
                            start=True, stop=True, skip_group_check=True)
                        e_t = etp.tile([P, 1024], BF16, tag="et")
                        nc.scalar.activation(
                            e_t[:], sp[:],
                            mybir.ActivationFunctionType.Exp, scale=0.125)
                        ets.append(e_t)
                    if kc >= AHEAD:
                        j = kc - AHEAD
                        nc.tensor.matmul(
                            pv_a[0:HD + 1, :], v_ext[:, j, h, :],
                            ets[j][:, 0:512],
                            start=(j == 0), stop=(j == SO - 1),
                            skip_group_check=True)
                        nc.tensor.matmul(
                            pv_b[0:HD + 1, :], v_ext[:, j, h, :],
                            ets[j][:, 512:1024],
                            start=(j == 0), stop=(j == SO - 1),
                            skip_group_check=True)
                # evict raw out + sums; broadcast sums (no PE dependency).
                # partition_broadcast only writes reliably at partition 0,
                # so odd heads bounce through a temp + DVE copy.
                for q2, pv in ((0, pv_a), (1, pv_b)):
                    qs = slice(q2 * 512, (q2 + 1) * 512)
                    nc.vector.tensor_copy(attn_c[hc][off:off + HD, qs],
                                          pv[0:HD, :])
                    srow = s2.tile([1, 512], F32, tag="s2")
                    nc.vector.tensor_copy(srow[:], pv[HD:HD + 1, :])
                    if off == 0:
                        nc.gpsimd.partition_broadcast(rb_c[0:HD, qs], srow[:])
                    else:
                        tmp = s2.tile([HD, 512], F32, tag="s2")
                        nc.gpsimd.partition_broadcast(tmp[:], srow[:])
                        nc.vector.tensor_copy(rb_c[off:off + HD, qs], tmp[:])
                if off == HD:
                    # chunk hc complete: odd head's rows + LN stats (all DVE)
                    rcp2 = s4.tile([P, S], F32, tag="s4")
                    nc.vector.reciprocal_approx_fast(rcp2[:], rb_c[:])
                    nc.vector.tensor_tensor(attn_c[hc][HD:P, :],
                                            attn_c[hc][HD:P, :],
                                            rcp2[HD:P, :],
                                            mybir.AluOpType.mult)
                    sq_c = sqp.tile([P, S], BF16, tag="sq")
                    nc.vector.tensor_tensor(sq_c[:], attn_c[hc][:],
                                            attn_c[hc][:],
                                            mybir.AluOpType.mult)
                    if hc == 0:
                        nc.vector.tensor_copy(acc[:], attn_c[0][:])
                        nc.vector.tensor_copy(acc2[:], sq_c[:])
                    elif hc < HO - 1:
                        nc.vector.tensor_tensor(acc[:], acc[:], attn_c[hc][:],
                                                mybir.AluOpType.add)
                        nc.vector.tensor_tensor(acc2[:], acc2[:], sq_c[:],
                                                mybir.AluOpType.add)
                    else:
                        sq7 = sq_c  # last chunk's stats go straight to PSUM

            # ---- phase 3: one block of transposed stats matmuls ([s-part,
            # so] layout) + LN scale factors -- all tiny ops
            stat_t = pvps.tile([P, 32], F32, tag="pv", name="stat_t")
            for so in range(SO):
                # chunks 0..6 ride acc/acc2 (hoistable off the tail); chunk 7
                # feeds its own columns directly so only these 16 tiny
                # matmuls trail the last normalize
                nc.tensor.matmul(
                    stat_t[:, so:so + 1], acc[:, so * P:(so + 1) * P],
                    ones_st[:], start=True, stop=True, skip_group_check=True)
                nc.tensor.matmul(
                    stat_t[:, 8 + so:9 + so], acc2[:, so * P:(so + 1) * P],
                    ones_st[:], start=True, stop=True, skip_group_check=True)
                nc.tensor.matmul(
                    stat_t[:, 16 + so:17 + so],
                    attn_c[HO - 1][:, so * P:(so + 1) * P],
                    ones_st[:], start=True, stop=True, skip_group_check=True)
                nc.tensor.matmul(
                    stat_t[:, 24 + so:25 + so], sq7[:, so * P:(so + 1) * P],
                    ones_st[:], start=True, stop=True, skip_group_check=True)
            acc16 = s2.tile([P, 16], F32, tag="s2")
            nc.vector.tensor_copy(acc16[:], stat_t[:, 0:16])
            nc.vector.tensor_tensor(acc16[:], acc16[:], stat_t[:, 16:32],
                                    mybir.AluOpType.add)
            m2 = s2.tile([P, 8], F32, tag="s2")
            nc.scalar.square(m2[:], acc16[:, 0:8])
            var_t = s2.tile([P, 8], F32, tag="s2")
            nc.vector.tensor_tensor(var_t[:], acc16[:, 8:16], m2[:],
                                    mybir.AluOpType.subtract)
            nc.scalar.activation(var_t[:], var_t[:],
                                 mybir.ActivationFunctionType.Sqrt, bias=eps_t[:])
            rstd = s2.tile([P, 8], F32, tag="s2")
            nc.vector.reciprocal_approx_fast(rstd[:], var_t[:])
            rmu = s2.tile([P, 8], F32, tag="s2")
            nc.vector.tensor_tensor(rmu[:], acc16[:, 0:8], rstd[:],
                                    mybir.AluOpType.mult)

            # ---- phase 4: output projection on raw attn with fused LN:
            # y = rstd*(attn^T @ W') - (mu*rstd)*g1 + b1
            for so in range(SO):
                for nh in range(2):
                    ps = mmps.tile([P, 512], F32, tag="mm")
                    for hc in range(HO):
                        nc.tensor.matmul(
                            ps[:], attn_c[hc][:, so * P:(so + 1) * P],
                            wo_t[:, hc, nh * 512:(nh + 1) * 512],
                            start=(hc == 0), stop=(hc == HO - 1))
                    t2 = s2.tile([P, 512], F32, tag="s2")
                    nc.vector.scalar_tensor_tensor(
                        t2[:], g1nb[:, nh * 512:(nh + 1) * 512],
                        rmu[:, so:so + 1], b1b[:, nh * 512:(nh + 1) * 512],
                        mybir.AluOpType.mult, mybir.AluOpType.add)
                    ych = s2.tile([P, 512], F32, tag="s2")
                    nc.vector.scalar_tensor_tensor(
                        ych[:], ps[:], rstd[:, so:so + 1], t2[:],
                        mybir.AluOpType.mult, mybir.AluOpType.add)
                    nc.sync.dma_start(
                        d_y.ap()[:, so, nh * 512:(nh + 1) * 512], ych[:])

    nc.compile()
    return nc


_NC = None


def _get_nc():
    global _NC
    if _NC is None:
        _NC = _build_program()
    return _NC


def _host_inputs(query, key, value, Wq, bq, Wk, bk, Wv, bv, Wo, bo,
                 temporal_weights, ln_gamma, ln_beta):
    T = _temporal_matrix(temporal_weights)
    tt_host = np.ascontiguousarray(  # TT[p, so, s'] = T[s', so*P+p]
        T.T.reshape(SO, P, S).transpose(1, 0, 2)).astype(NPBF16)
    cos, sin = _rope_tables()
    gam = np.asarray(ln_gamma, np.float32)
    bet = np.asarray(ln_beta, np.float32)
    Wo32 = np.asarray(Wo, np.float32)
    Wfold = gam[:, None] * Wo32               # gamma folded into out-proj
    g1n = -(gam @ Wo32).reshape(1, H)         # rank-1 LN correction row
    b1 = (bet @ Wo32 + np.asarray(bo, np.float32)).reshape(1, H)
    common = {
        "w_v": _nat(np.asarray(Wv, np.float32)).astype(NPBF16),
        "w_q": _nat(np.asarray(Wq, np.float32)).astype(NPBF16),
        "w_k": _nat(np.asarray(Wk, np.float32)).astype(NPBF16),
        "w_o": _nat(Wfold).astype(NPBF16),
        "b_v": np.asarray(bv, np.float32).reshape(1, H),
        "b_q": np.asarray(bq, np.float32).reshape(1, H),
        "b_k": np.asarray(bk, np.float32).reshape(1, H),
        "b_o": b1,
        "g1n": g1n,
        "tt": tt_host,
        "zpad": np.zeros((HD, S), NPBF16),
        "cos_t": _nat(cos).astype(NPBF16),
        "sin_t": _nat(sin).astype(NPBF16),
    }
    in_maps = []
    for c in range(N_CORES):
        m = dict(common)
        m["xt_q"] = _xt_chunks(np.asarray(query[c], np.float32)).astype(NPBF16)
        m["xt_k"] = _xt_chunks(np.asarray(key[c], np.float32)).astype(NPBF16)
        m["xt_v"] = _xt_chunks(np.asarray(value[c], np.float32)).astype(NPBF16)
        in_maps.append(m)
    return in_maps


def kernel(query, key, value, Wq, bq, Wk, bk, Wv, bv, Wo, bo,
           temporal_weights, ln_gamma, ln_beta):
    in_maps = _host_inputs(query, key, value, Wq, bq, Wk, bk, Wv, bv, Wo, bo,
                           temporal_weights, ln_gamma, ln_beta)
    nc = _get_nc()
    res = run_bass_kernel_spmd(nc, in_maps, list(range(N_CORES)))
    out = np.empty((B, S, H), np.float32)
    for c in range(N_CORES):
        y = res.results[c]["y"]  # [P, SO, H]
        out[c] = y.transpose(1, 0, 2).reshape(S, H)
    return out


# revision 20
# speedup vs baseline: 1.0211x; 1.0001x over previous
"""HSTU-style attention block (RoPE + multi-scale temporal agg + SDPA + LN + out-proj)
for Trainium2, data-parallel over batch across 8 NeuronCores.

Per-core layout strategy (batch element per core):
  - host pre-transposes X so projections run with activations as lhsT
  - Q/K/V projected into natural [s, h'] layout; RoPE applied in-place in bf16
    (all-bf16 packed operands ride the DVE 2x mode)
  - temporal aggregation applied as a matmul against a host-built [S, S] matrix T
    (softmax(temporal_weights)); band structure (|s'-s| <= 11) trims contraction
    chunks at 256-wide output granularity; Q/K produced transposed, V natural
    with an extra ones column so softmax denominators ride the PV matmul
  - attention computes scores^T per head over the FULL query range: two N=512
    matmuls land in one two-bank [128,1024] PSUM tile so a single scalar-engine
    Exp covers each key chunk (the Exp stream is the phase-2 floor), pipelined
    AHEAD chunks in front of the PV accumulation
  - LayerNorm statistics accumulate on the DVE in bf16 (sum / sum-of-squares
    per chunk); one block of 16 transposed N=1 ones-matmuls at the end of
    phase 2 moves them into the [s-partition] layout, where gamma/beta fold
    into the out-projection weights (host) and the LN apply collapses to a
    per-partition scale at PSUM eviction plus a rank-1 mu*rstd correction
All matmuls run in bfloat16 (fp32 PSUM accumulation).
"""

import numpy as np
import ml_dtypes
import concourse.mybir as mybir
import concourse.tile as tile
from concourse import bacc
from concourse.bass_utils import run_bass_kernel_spmd

B, S, H, NH = 8, 1024, 1024, 16
HD = H // NH  # 64
P = 128
SO = S // P  # 8
HO = H // P  # 8
N_SCALES = 4
LN_EPS = 1e-5
F32 = mybir.dt.float32
BF16 = mybir.dt.bfloat16
NPBF16 = ml_dtypes.bfloat16

N_CORES = 8
AHEAD = 2  # exp pipeline depth (score chunks ahead of PV)


# ---------------------------------------------------------------- host helpers
def _softmax_np(x):
    x = np.asarray(x, np.float64)
    e = np.exp(x - x.max())
    return e / e.sum()


def _temporal_matrix(temporal_weights):
    """[S, S] matrix T with (T @ x) == temporal_agg(x) along the sequence axis."""
    w = _softmax_np(temporal_weights)
    T = np.eye(S, dtype=np.float64) * w[0]
    for scale in range(1, N_SCALES):
        p = max(1, S // (2 ** scale))
        k = S // p
        pool = np.zeros((p, S), dtype=np.float64)
        for j in range(p):
            pool[j, j * k:(j + 1) * k] = 1.0 / k
        coord = (np.arange(S, dtype=np.float64) + 0.5) * (p / S) - 0.5
        coord = np.clip(coord, 0.0, None)
        i0 = np.minimum(np.floor(coord).astype(np.int64), p - 1)
        i1 = np.minimum(i0 + 1, p - 1)
        lam = (coord - i0).astype(np.float32).astype(np.float64)
        interp = np.zeros((S, p), dtype=np.float64)
        interp[np.arange(S), i0] += 1.0 - lam
        interp[np.arange(S), i1] += lam
        T += w[scale] * (interp @ pool)
    return T.astype(np.float32)


def _rope_tables():
    inv_freq = 1.0 / (10000.0 ** (np.arange(0, HD, 2, dtype=np.float64) / HD))
    freqs = np.arange(S, dtype=np.float64)[:, None] * inv_freq[None, :]
    cos = np.repeat(np.cos(freqs), 2, axis=-1).astype(np.float32)  # [S, HD]
    sin = np.repeat(np.sin(freqs), 2, axis=-1).astype(np.float32)
    return cos, sin


def _nat(x):
    """[S, D] -> [P, S//P, D] with x[so*P+p, d] = out[p, so, d]."""
    return np.ascontiguousarray(x.reshape(SO, P, x.shape[-1]).transpose(1, 0, 2))


def _xt_chunks(x):
    """[S, H] -> [P, SO, HO*P] with out[p, so, ho*P + i] = x[so*P + i, ho*P + p]."""
    return np.ascontiguousarray(
        x.reshape(SO, P, HO, P).transpose(3, 0, 2, 1).reshape(P, SO, H))


# ---------------------------------------------------------------- bass program
def _build_program():
    nc = bacc.Bacc("TRN2", target_bir_lowering=False, debug=False)

    d_xt = {a: nc.dram_tensor(f"xt_{a}", [P, SO, H], BF16, kind="ExternalInput")
            for a in ("v", "q", "k")}
    d_w = {a: nc.dram_tensor(f"w_{a}", [P, HO, H], BF16, kind="ExternalInput")
           for a in ("v", "q", "k", "o")}
    d_b = {a: nc.dram_tensor(f"b_{a}", [1, H], F32, kind="ExternalInput")
           for a in ("v", "q", "k", "o")}
    d_g1n = nc.dram_tensor("g1n", [1, H], F32, kind="ExternalInput")
    d_tt = nc.dram_tensor("tt", [P, SO, S], BF16, kind="ExternalInput")
    d_cos = nc.dram_tensor("cos_t", [P, SO, HD], BF16, kind="ExternalInput")
    d_sin = nc.dram_tensor("sin_t", [P, SO, HD], BF16, kind="ExternalInput")
    d_y = nc.dram_tensor("y", [P, SO, H], F32, kind="ExternalOutput")
    d_zp = nc.dram_tensor("zpad", [HD, S], BF16, kind="ExternalInput")
    # per-chunk scratch so a head's reload only waits on its own spill DMA
    d_qs = [nc.dram_tensor(f"q_scr{hc}", [P, S], BF16) for hc in range(HO)]
    d_ks = [nc.dram_tensor(f"k_scr{hc}", [P, S], BF16) for hc in range(HO)]

    with tile.TileContext(nc) as tc:
        with (
            tc.tile_pool(name="const", bufs=1) as cpool,
            tc.tile_pool(name="big", bufs=5) as big,
            tc.tile_pool(name="s4", bufs=6) as s4,
            tc.tile_pool(name="xt", bufs=3) as xtp,
            tc.tile_pool(name="rot", bufs=2) as rotp,
            tc.tile_pool(name="kq", bufs=6) as kqp,
            tc.tile_pool(name="s2", bufs=6) as s2,
            tc.tile_pool(name="et", bufs=4) as etp,
            tc.tile_pool(name="sq", bufs=2) as sqp,
            tc.tile_pool(name="at", bufs=8) as atp,
            tc.tile_pool(name="acc", bufs=2) as accp,
            tc.tile_pool(name="mm_ps", bufs=2, space="PSUM") as mmps,
            tc.tile_pool(name="sc_ps", bufs=2, space="PSUM") as scps,
            tc.tile_pool(name="pv_ps", bufs=2, space="PSUM") as pvps,
        ):
            cos_t = cpool.tile([P, SO, HD], BF16, name="cos_t")
            sin_t = cpool.tile([P, SO, HD], BF16, name="sin_t")
            nc.sync.dma_start(cos_t[:], d_cos.ap())
            nc.sync.dma_start(sin_t[:], d_sin.ap())
            ones = cpool.tile([P, 1], F32, name="ones")
            nc.vector.memset(ones[:], 1.0)
            eps_t = cpool.tile([P, 1], F32, name="eps_t")
            nc.vector.memset(eps_t[:], LN_EPS)
            sqwarm = cpool.tile([P, 1], F32, name="sqwarm")
            nc.scalar.activation(sqwarm[:], eps_t[:],
                                 mybir.ActivationFunctionType.Sqrt)
            # stats rhs: 1/H so the PSUM accumulators hold means directly
            ones_st = cpool.tile([P, 1], BF16, name="ones_st")
            nc.vector.memset(ones_st[:], 1.0 / H)

            def _rope_chunk(a_nat, so):
                """In-place bf16 RoPE on a_nat[:, so, :] (DVE 2x mode)."""
                ch = a_nat[:, so, :]
                ch3 = ch.rearrange("p (nh d) -> p nh d", d=HD)
                ch4 = ch.rearrange("p (nh hf dd) -> p nh hf dd", hf=2, dd=HD // 2)
                rot = rotp.tile([P, H], BF16, tag="rot")
                rot4 = rot[:].rearrange("p (nh hf dd) -> p nh hf dd",
                                        hf=2, dd=HD // 2)
                rot3 = rot[:].rearrange("p (nh d) -> p nh d", d=HD)
                nc.vector.tensor_scalar_mul(rot4[:, :, 0, :], ch4[:, :, 1, :], -1.0)
                nc.vector.tensor_copy(rot4[:, :, 1, :], ch4[:, :, 0, :])
                cb = cos_t[:, so, :][:, None, :].to_broadcast((P, NH, HD))
                sb = sin_t[:, so, :][:, None, :].to_broadcast((P, NH, HD))
                nc.vector.tensor_tensor(ch3[:], ch3[:], cb, mybir.AluOpType.mult)
                nc.vector.tensor_tensor(rot3[:], rot3[:], sb, mybir.AluOpType.mult)
                nc.vector.tensor_tensor(ch[:], ch[:], rot[:], mybir.AluOpType.add)

            def project(a, do_rope=False):
                """A_nat [P, SO, H] (bf16) = X @ W_a + b_a, optional fused RoPE."""
                w_t = big.tile([P, HO, H], BF16, tag="big")
                for ko in range(HO):
                    nc.sync.dma_start(w_t[:, ko, :], d_w[a].ap()[:, ko, :])
                brow = s4.tile([1, H], F32, tag="s4")
                nc.sync.dma_start(brow[:], d_b[a].ap())
                bb = s4.tile([P, H], F32, tag="s4")
                nc.gpsimd.partition_broadcast(bb[:], brow[:])
                a_nat = big.tile([P, SO, H], BF16, tag="big")
                for so in range(SO):
                    xt_c = xtp.tile([P, HO, P], BF16, tag="xt")
                    nc.sync.dma_start(xt_c[:], d_xt[a].ap()[:, so, :])
                    for nh in range(2):
                        ps = mmps.tile([P, 512], F32, tag="mm")
                        for ko in range(HO):
                            nc.tensor.matmul(
                                ps[:], xt_c[:, ko, :],
                                w_t[:, ko, nh * 512:(nh + 1) * 512],
                                start=(ko == 0), stop=(ko == HO - 1))
                        nc.vector.tensor_tensor(
                            a_nat[:, so, nh * 512:(nh + 1) * 512], ps[:],
                            bb[:, nh * 512:(nh + 1) * 512], mybir.AluOpType.add)
                    if do_rope:
                        _rope_chunk(a_nat, so)
                return a_nat

            BAND = 12  # T[s', s] == 0 for |s' - s| > 11 (structural)

            def _band_sos(o0, o1):
                """so chunks whose s-range intersects [o0-BAND, o1+BAND)."""
                return [so for so in range(SO)
                        if so * P + P > o0 - BAND and so * P < o1 + BAND]

            def spill_chunk(a_nat, tt, hc, d_scr):
                """One h'-chunk of (T @ A).T spilled to DRAM scratch. Runs
                interleaved with attention, so the eviction rides the DVE
                (the scalar engine is saturated by the Exp stream there)."""
                for sh in range(2):
                    ps = mmps.tile([P, 512], F32, tag="mm")
                    for q in range(2):
                        o0 = sh * 512 + q * 256
                        sos = _band_sos(o0, o0 + 256)
                        for so in sos:
                            nc.tensor.matmul(
                                ps[:, q * 256:(q + 1) * 256],
                                a_nat[:, so, hc * P:(hc + 1) * P],
                                tt[:, so, o0:o0 + 256],
                                start=(so == sos[0]), stop=(so == sos[-1]),
                                skip_group_check=True)
                    ev = s2.tile([P, 512], BF16, tag="s2")
                    nc.vector.tensor_copy(ev[:], ps[:])
                    nc.sync.dma_start(
                        d_scr[hc].ap()[:, sh * 512:(sh + 1) * 512], ev[:])

            def t_agg_v(v_nat, tt):
                """V_ext [P, SO, NH, HD+1] (bf16) = T @ V with ones column."""
                v_ext = big.tile([P, SO, NH, HD + 1], BF16, tag="big")
                nc.vector.tensor_copy(
                    v_ext[:, :, :, HD:HD + 1],
                    ones[:, None, None, :].to_broadcast((P, SO, NH, 1)))
                for sc in range(SO):
                    sos = _band_sos(sc * P, (sc + 1) * P)
                    for dh in range(2):
                        ps = mmps.tile([P, 512], F32, tag="mm")
                        for so in sos:
                            nc.tensor.matmul(
                                ps[:], tt[:, so, sc * P:(sc + 1) * P],
                                v_nat[:, so, dh * 512:(dh + 1) * 512],
                                start=(so == sos[0]), stop=(so == sos[-1]))
                        pvw = ps[:].rearrange("p (nh d) -> p nh d", d=HD)
                        nc.scalar.copy(
                            v_ext[:, sc, dh * 8:(dh + 1) * 8, 0:HD], pvw)
                return v_ext

            # ---- phase 1: V, Q, K  (projection + RoPE + temporal aggregation)
            v_nat = project("v")
            tt = big.tile([P, SO, S], BF16, tag="big")
            nc.sync.dma_start(tt[:], d_tt.ap())
            v_ext = t_agg_v(v_nat, tt)

            q_nat = project("q", do_rope=True)
            k_nat = project("k", do_rope=True)

            pre_kq = {}
            # six persistent reload buffers: rows 64:128 are zeroed once and
            # never rewritten (reload DMAs only touch rows 0:64), so the
            # K=128 zero-padding costs no per-head DMA traffic
            kq_bufs = [kqp.tile([P, S], BF16, tag="kq", name=f"kqb{i}")
                       for i in range(6)]
            for b in kq_bufs:
                nc.vector.memset(b[HD:P, :], 0.0)

            def _load_head(h):
                off = (h % 2) * HD
                hc = h // 2
                kh = kq_bufs[(2 * h) % 6]
                nc.sync.dma_start(kh[0:HD, :], d_ks[hc].ap()[off:off + HD, :])
                qh = kq_bufs[(2 * h + 1) % 6]
                nc.sync.dma_start(qh[0:HD, :], d_qs[hc].ap()[off:off + HD, :])
                return kh, qh

            # spill the first two h'-chunks of Q/K aggregation upfront; the
            # remaining chunks interleave with the head loop two chunks ahead
            for hc0 in (0, 1):
                spill_chunk(q_nat, tt, hc0, d_qs)
                spill_chunk(k_nat, tt, hc0, d_ks)
            pre_kq[0] = _load_head(0)
            pre_kq[1] = _load_head(1)

            # prefetch out-projection weights + folded LN rows during phase 2
            wo_t = big.tile([P, HO, H], BF16, tag="big")
            nc.sync.dma_start(wo_t[:], d_w["o"].ap())
            b1r = s4.tile([1, H], F32, tag="s4")
            nc.sync.dma_start(b1r[:], d_b["o"].ap())
            b1b = cpool.tile([P, H], F32, name="b1b")
            nc.gpsimd.partition_broadcast(b1b[:], b1r[:])
            g1r = s4.tile([1, H], F32, tag="s4")
            nc.sync.dma_start(g1r[:], d_g1n.ap())
            g1nb = cpool.tile([P, H], F32, name="g1nb")
            nc.gpsimd.partition_broadcast(g1nb[:], g1r[:])

            # ---- phase 2: attention. One attention-out tile per h' chunk
            # (separate tensors keep the scheduler from inventing cross-chunk
            # dependencies); LN stats accumulate on the DVE only.
            attn_c = [atp.tile([P, S], BF16, tag="at", name=f"attn{c}")
                      for c in range(HO)]
            acc = accp.tile([P, S], BF16, tag="acc", name="acc")
            acc2 = accp.tile([P, S], BF16, tag="acc", name="acc2")

            rb_c = None
            for h in range(NH):
                hc, off = h // 2, (h % 2) * HD
                # zero-pad the contraction dim to K=128 (rows 64:128 from a
                # DRAM zeros pad) to keep the PE activity profile flat
                if h in pre_kq:
                    kh, qh = pre_kq[h]
                else:
                    kh, qh = _load_head(h)
                if off == 0:
                    if hc + 2 < HO:
                        spill_chunk(q_nat, tt, hc + 2, d_qs)
                        spill_chunk(k_nat, tt, hc + 2, d_ks)
                    rb_c = s4.tile([P, S], F32, tag="s4")
                    rcp_c = s4.tile([P, S], F32, tag="s4")
                else:
                    # even head's rows normalize while the odd head computes,
                    # halving the serial chain at every chunk boundary. The
                    # custom reciprocal op must start at partition 0 (its
                    # seed constants are partition-aligned), so it runs on
                    # the full tile; the top half is recomputed at the
                    # boundary once the odd head's denominators land.
                    nc.vector.reciprocal_approx_fast(rcp_c[:], rb_c[:])
                    nc.vector.tensor_tensor(attn_c[hc][0:HD, :],
                                            attn_c[hc][0:HD, :],
                                            rcp_c[0:HD, :],
                                            mybir.AluOpType.mult)
                pv_a = pvps.tile([P, 512], F32, tag="pv", name="pv_a")
                pv_b = pvps.tile([P, 512], F32, tag="pv", name="pv_b")
                # software-pipelined: both query halves of a key chunk land in
                # one two-bank PSUM tile so a single Exp serves the chunk,
                # running AHEAD chunks in front of the PV accumulation
                ets = []
                for kc in range(SO + AHEAD):
                    if kc < SO:
                        sp = scps.tile([P, 1024], F32, tag="sc")
                        nc.tensor.matmul(
                            sp[:, 0:512], kh[0:P, kc * P:(kc + 1) * P],
                            qh[0:P, 0:512],
                            start=True, stop=True, skip_group_check=True)
                        nc.tensor.matmul(
                            sp[:, 512:1024], kh[0:P, kc * P:(kc + 1) * P],
                            qh[0:P, 512:1024],
                            start=True, stop=True, skip_group_check=True)
                        e_t = etp.tile([P, 1024], BF16, tag="et")
                        nc.scalar.activation(
                            e_t[:], sp[:],
                            mybir.ActivationFunctionType.Exp, scale=0.125)
                        ets.append(e_t)
                    if kc >= AHEAD:
                        j = kc - AHEAD
                        nc.tensor.matmul(
                            pv_a[0:HD + 1, :], v_ext[:, j, h, :],
                            ets[j][:, 0:512],
                            start=(j == 0), stop=(j == SO - 1),
                            skip_group_check=True)
                        nc.tensor.matmul(
                            pv_b[0:HD + 1, :], v_ext[:, j, h, :],
                            ets[j][:, 512:1024],
                            start=(j == 0), stop=(j == SO - 1),
                            skip_group_check=True)
                # evict raw out + sums; broadcast sums (no PE dependency).
                # partition_broadcast only writes reliably at partition 0,
                # so odd heads bounce through a temp + DVE copy.
                for q2, pv in ((0, pv_a), (1, pv_b)):
                    qs = slice(q2 * 512, (q2 + 1) * 512)
                    nc.vector.tensor_copy(attn_c[hc][off:off + HD, qs],
                                          pv[0:HD, :])
                    srow = s2.tile([1, 512], F32, tag="s2")
                    nc.vector.tensor_copy(srow[:], pv[HD:HD + 1, :])
                    if off == 0:
                        nc.gpsimd.partition_broadcast(rb_c[0:HD, qs], srow[:])
                    else:
                        tmp = s2.tile([HD, 512], F32, tag="s2")
                        nc.gpsimd.partition_broadcast(tmp[:], srow[:])
                        nc.vector.tensor_copy(rb_c[off:off + HD, qs], tmp[:])
                if off == HD:
                    # chunk hc complete: odd head's rows + LN stats (all DVE)
                    rcp2 = s4.tile([P, S], F32, tag="s4")
                    nc.vector.reciprocal_approx_fast(rcp2[:], rb_c[:])
                    nc.vector.tensor_tensor(attn_c[hc][HD:P, :],
                                            attn_c[hc][HD:P, :],
                                            rcp2[HD:P, :],
                                            mybir.AluOpType.mult)
                    sq_c = sqp.tile([P, S], BF16, tag="sq")
                    nc.vector.tensor_tensor(sq_c[:], attn_c[hc][:],
                                            attn_c[hc][:],
                                            mybir.AluOpType.mult)
                    if hc == 0:
                        nc.vector.tensor_copy(acc[:], attn_c[0][:])
                        nc.vector.tensor_copy(acc2[:], sq_c[:])
                    elif hc < HO - 1:
                        nc.vector.tensor_tensor(acc[:], acc[:], attn_c[hc][:],
                                                mybir.AluOpType.add)
                        nc.vector.tensor_tensor(acc2[:], acc2[:], sq_c[:],
                                                mybir.AluOpType.add)
                    else:
                        sq7 = sq_c  # last chunk's stats go straight to PSUM

            # out-proj partial chains for the first two (so, nh) pairs:
            # chunks 0..6 are ready one boundary early, so these fill the PE
            # while the last chunk's normalize chain runs on the DVE
            pairs = [(so, nh) for so in range(SO) for nh in range(2)]
            pair_ps = []
            for so, nh in pairs[:2]:
                psp = mmps.tile([P, 512], F32, tag="mm", name="psp")
                pair_ps.append(psp)
                for hc in range(HO - 1):
                    nc.tensor.matmul(
                        psp[:], attn_c[hc][:, so * P:(so + 1) * P],
                        wo_t[:, hc, nh * 512:(nh + 1) * 512],
                        start=(hc == 0), stop=False)

            # ---- phase 3: one block of transposed stats matmuls ([s-part,
            # so] layout) + LN scale factors -- all tiny ops
            stat_t = pvps.tile([P, 32], F32, tag="pv", name="stat_t")
            for so in range(SO):
                # chunks 0..6 ride acc/acc2 (hoistable off the tail); chunk 7
                # feeds its own columns directly so only these 16 tiny
                # matmuls trail the last normalize
                nc.tensor.matmul(
                    stat_t[:, so:so + 1], acc[:, so * P:(so + 1) * P],
                    ones_st[:], start=True, stop=True, skip_group_check=True)
                nc.tensor.matmul(
                    stat_t[:, 8 + so:9 + so], acc2[:, so * P:(so + 1) * P],
                    ones_st[:], start=True, stop=True, skip_group_check=True)
                nc.tensor.matmul(
                    stat_t[:, 16 + so:17 + so],
                    attn_c[HO - 1][:, so * P:(so + 1) * P],
                    ones_st[:], start=True, stop=True, skip_group_check=True)
                nc.tensor.matmul(
                    stat_t[:, 24 + so:25 + so], sq7[:, so * P:(so + 1) * P],
                    ones_st[:], start=True, stop=True, skip_group_check=True)
            acc16 = s2.tile([P, 16], F32, tag="s2")
            nc.vector.tensor_copy(acc16[:], stat_t[:, 0:16])
            nc.vector.tensor_tensor(acc16[:], acc16[:], stat_t[:, 16:32],
                                    mybir.AluOpType.add)
            m2 = s2.tile([P, 8], F32, tag="s2")
            nc.scalar.square(m2[:], acc16[:, 0:8])
            var_t = s2.tile([P, 8], F32, tag="s2")
            nc.vector.tensor_tensor(var_t[:], acc16[:, 8:16], m2[:],
                                    mybir.AluOpType.subtract)
            nc.scalar.activation(var_t[:], var_t[:],
                                 mybir.ActivationFunctionType.Sqrt, bias=eps_t[:])
            rstd = s2.tile([P, 8], F32, tag="s2")
            nc.vector.reciprocal_approx_fast(rstd[:], var_t[:])
            rmu = s2.tile([P, 8], F32, tag="s2")
            nc.vector.tensor_tensor(rmu[:], acc16[:, 0:8], rstd[:],
                                    mybir.AluOpType.mult)

            # ---- phase 4: output projection on raw attn with fused LN:
            # y = rstd*(attn^T @ W') - (mu*rstd)*g1 + b1. The first two
            # pairs' chunk-0..6 partial chains were issued before the stats
            # block (above), filling the PE while the last normalize runs.
            for i, (so, nh) in enumerate(pairs):
                ps = pair_ps[i] if i < 2 else mmps.tile([P, 512], F32,
                                                        tag="mm", name="ps")
                h0 = HO - 1 if i < 2 else 0
                for hc in range(h0, HO):
                    nc.tensor.matmul(
                        ps[:], attn_c[hc][:, so * P:(so + 1) * P],
                        wo_t[:, hc, nh * 512:(nh + 1) * 512],
                        start=(hc == 0), stop=(hc == HO - 1))
                t2 = s2.tile([P, 512], F32, tag="s2")
                nc.vector.scalar_tensor_tensor(
                    t2[:], g1nb[:, nh * 512:(nh + 1) * 512],
                    rmu[:, so:so + 1], b1b[:, nh * 512:(nh + 1) * 512],
                    mybir.AluOpType.mult, mybir.AluOpType.add)
                ych = s2.tile([P, 512], F32, tag="s2")
                nc.vector.scalar_tensor_tensor(
                    ych[:], ps[:], rstd[:, so:so + 1], t2[:],
                    mybir.AluOpType.mult, mybir.AluOpType.add)
                nc.sync.dma_start(
                    d_y.ap()[:, so, nh * 512:(nh + 1) * 512], ych[:])

    nc.compile()
    return nc


_NC = None


def _get_nc():
    global _NC
    if _NC is None:
        _NC = _build_program()
    return _NC


def _host_inputs(query, key, value, Wq, bq, Wk, bk, Wv, bv, Wo, bo,
                 temporal_weights, ln_gamma, ln_beta):
    T = _temporal_matrix(temporal_weights)
    tt_host = np.ascontiguousarray(  # TT[p, so, s'] = T[s', so*P+p]
        T.T.reshape(SO, P, S).transpose(1, 0, 2)).astype(NPBF16)
    cos, sin = _rope_tables()
    gam = np.asarray(ln_gamma, np.float32)
    bet = np.asarray(ln_beta, np.float32)
    Wo32 = np.asarray(Wo, np.float32)
    Wfold = gam[:, None] * Wo32               # gamma folded into out-proj
    g1n = -(gam @ Wo32).reshape(1, H)         # rank-1 LN correction row
    b1 = (bet @ Wo32 + np.asarray(bo, np.float32)).reshape(1, H)
    common = {
        "w_v": _nat(np.asarray(Wv, np.float32)).astype(NPBF16),
        "w_q": _nat(np.asarray(Wq, np.float32)).astype(NPBF16),
        "w_k": _nat(np.asarray(Wk, np.float32)).astype(NPBF16),
        "w_o": _nat(Wfold).astype(NPBF16),
        "b_v": np.asarray(bv, np.float32).reshape(1, H),
        "b_q": np.asarray(bq, np.float32).reshape(1, H),
        "b_k": np.asarray(bk, np.float32).reshape(1, H),
        "b_o": b1,
        "g1n": g1n,
        "tt": tt_host,
        "zpad": np.zeros((HD, S), NPBF16),
        "cos_t": _nat(cos).astype(NPBF16),
        "sin_t": _nat(sin).astype(NPBF16),
    }
    in_maps = []
    for c in range(N_CORES):
        m = dict(common)
        m["xt_q"] = _xt_chunks(np.asarray(query[c], np.float32)).astype(NPBF16)
        m["xt_k"] = _xt_chunks(np.asarray(key[c], np.float32)).astype(NPBF16)
        m["xt_v"] = _xt_chunks(np.asarray(value[c], np.float32)).astype(NPBF16)
        in_maps.append(m)
    return in_maps


def kernel(query, key, value, Wq, bq, Wk, bk, Wv, bv, Wo, bo,
           temporal_weights, ln_gamma, ln_beta):
    in_maps = _host_inputs(query, key, value, Wq, bq, Wk, bk, Wv, bv, Wo, bo,
                           temporal_weights, ln_gamma, ln_beta)
    nc = _get_nc()
    res = run_bass_kernel_spmd(nc, in_maps, list(range(N_CORES)))
    out = np.empty((B, S, H), np.float32)
    for c in range(N_CORES):
        y = res.results[c]["y"]  # [P, SO, H]
        out[c] = y.transpose(1, 0, 2).reshape(S, H)
    return out


# revision 21
# speedup vs baseline: 1.0375x; 1.0161x over previous
"""HSTU-style attention block (RoPE + multi-scale temporal agg + SDPA + LN + out-proj)
for Trainium2, data-parallel over batch across 8 NeuronCores.

Per-core layout strategy (batch element per core):
  - host pre-transposes X so projections run with activations as lhsT
  - Q/K/V projected into natural [s, h'] layout; RoPE applied in-place in bf16
    (all-bf16 packed operands ride the DVE 2x mode)
  - temporal aggregation applied as a matmul against a host-built [S, S] matrix T
    (softmax(temporal_weights)); band structure (|s'-s| <= 11) trims contraction
    chunks at 256-wide output granularity; Q/K produced transposed, V natural
    with an extra ones column so softmax denominators ride the PV matmul
  - attention computes scores^T per head over the FULL query range: two N=512
    matmuls land in one two-bank [128,1024] PSUM tile so a single scalar-engine
    Exp covers each key chunk (the Exp stream is the phase-2 floor), pipelined
    AHEAD chunks in front of the PV accumulation
  - LayerNorm statistics accumulate on the DVE in bf16 (sum / sum-of-squares
    per chunk); one block of 16 transposed N=1 ones-matmuls at the end of
    phase 2 moves them into the [s-partition] layout, where gamma/beta fold
    into the out-projection weights (host) and the LN apply collapses to a
    per-partition scale at PSUM eviction plus a rank-1 mu*rstd correction
All matmuls run in bfloat16 (fp32 PSUM accumulation).
"""

import numpy as np
import ml_dtypes
import concourse.mybir as mybir
import concourse.tile as tile
from concourse import bacc
from concourse.bass_utils import run_bass_kernel_spmd

B, S, H, NH = 8, 1024, 1024, 16
HD = H // NH  # 64
P = 128
SO = S // P  # 8
HO = H // P  # 8
N_SCALES = 4
LN_EPS = 1e-5
F32 = mybir.dt.float32
BF16 = mybir.dt.bfloat16
NPBF16 = ml_dtypes.bfloat16

N_CORES = 8
AHEAD = 3  # exp pipeline depth (score chunks ahead of PV)


# ---------------------------------------------------------------- host helpers
def _softmax_np(x):
    x = np.asarray(x, np.float64)
    e = np.exp(x - x.max())
    return e / e.sum()


def _temporal_matrix(temporal_weights):
    """[S, S] matrix T with (T @ x) == temporal_agg(x) along the sequence axis."""
    w = _softmax_np(temporal_weights)
    T = np.eye(S, dtype=np.float64) * w[0]
    for scale in range(1, N_SCALES):
        p = max(1, S // (2 ** scale))
        k = S // p
        pool = np.zeros((p, S), dtype=np.float64)
        for j in range(p):
            pool[j, j * k:(j + 1) * k] = 1.0 / k
        coord = (np.arange(S, dtype=np.float64) + 0.5) * (p / S) - 0.5
        coord = np.clip(coord, 0.0, None)
        i0 = np.minimum(np.floor(coord).astype(np.int64), p - 1)
        i1 = np.minimum(i0 + 1, p - 1)
        lam = (coord - i0).astype(np.float32).astype(np.float64)
        interp = np.zeros((S, p), dtype=np.float64)
        interp[np.arange(S), i0] += 1.0 - lam
        interp[np.arange(S), i1] += lam
        T += w[scale] * (interp @ pool)
    return T.astype(np.float32)


def _rope_tables():
    inv_freq = 1.0 / (10000.0 ** (np.arange(0, HD, 2, dtype=np.float64) / HD))
    freqs = np.arange(S, dtype=np.float64)[:, None] * inv_freq[None, :]
    cos = np.repeat(np.cos(freqs), 2, axis=-1).astype(np.float32)  # [S, HD]
    sin = np.repeat(np.sin(freqs), 2, axis=-1).astype(np.float32)
    return cos, sin


def _nat(x):
    """[S, D] -> [P, S//P, D] with x[so*P+p, d] = out[p, so, d]."""
    return np.ascontiguousarray(x.reshape(SO, P, x.shape[-1]).transpose(1, 0, 2))


def _xt_chunks(x):
    """[S, H] -> [P, SO, HO*P] with out[p, so, ho*P + i] = x[so*P + i, ho*P + p]."""
    return np.ascontiguousarray(
        x.reshape(SO, P, HO, P).transpose(3, 0, 2, 1).reshape(P, SO, H))


# ---------------------------------------------------------------- bass program
def _build_program():
    nc = bacc.Bacc("TRN2", target_bir_lowering=False, debug=False)

    d_xt = {a: nc.dram_tensor(f"xt_{a}", [P, SO, H], BF16, kind="ExternalInput")
            for a in ("v", "q", "k")}
    d_w = {a: nc.dram_tensor(f"w_{a}", [P, HO, H], BF16, kind="ExternalInput")
           for a in ("v", "q", "k", "o")}
    d_b = {a: nc.dram_tensor(f"b_{a}", [1, H], F32, kind="ExternalInput")
           for a in ("v", "q", "k", "o")}
    d_g1n = nc.dram_tensor("g1n", [1, H], F32, kind="ExternalInput")
    d_tt = nc.dram_tensor("tt", [P, SO, S], BF16, kind="ExternalInput")
    d_cos = nc.dram_tensor("cos_t", [P, SO, HD], BF16, kind="ExternalInput")
    d_sin = nc.dram_tensor("sin_t", [P, SO, HD], BF16, kind="ExternalInput")
    d_y = nc.dram_tensor("y", [P, SO, H], F32, kind="ExternalOutput")
    d_zp = nc.dram_tensor("zpad", [HD, S], BF16, kind="ExternalInput")
    # per-chunk scratch so a head's reload only waits on its own spill DMA
    d_qs = [nc.dram_tensor(f"q_scr{hc}", [P, S], BF16) for hc in range(HO)]
    d_ks = [nc.dram_tensor(f"k_scr{hc}", [P, S], BF16) for hc in range(HO)]

    with tile.TileContext(nc) as tc:
        with (
            tc.tile_pool(name="const", bufs=1) as cpool,
            tc.tile_pool(name="big", bufs=5) as big,
            tc.tile_pool(name="s4", bufs=6) as s4,
            tc.tile_pool(name="xt", bufs=3) as xtp,
            tc.tile_pool(name="rot", bufs=2) as rotp,
            tc.tile_pool(name="kq", bufs=6) as kqp,
            tc.tile_pool(name="s2", bufs=6) as s2,
            tc.tile_pool(name="et", bufs=5) as etp,
            tc.tile_pool(name="sq", bufs=2) as sqp,
            tc.tile_pool(name="at", bufs=8) as atp,
            tc.tile_pool(name="acc", bufs=2) as accp,
            tc.tile_pool(name="mm_ps", bufs=2, space="PSUM") as mmps,
            tc.tile_pool(name="sc_ps", bufs=2, space="PSUM") as scps,
            tc.tile_pool(name="pv_ps", bufs=2, space="PSUM") as pvps,
        ):
            # the first matmul needs xt_v chunk 0 + w_v chunk 0: issue them
            # before anything else (the DMA feeder runs in program order)
            w_v0 = big.tile([P, HO, H], BF16, tag="big", name="w_v0")
            nc.sync.dma_start(w_v0[:, 0, :], d_w["v"].ap()[:, 0, :])
            xt_v0 = xtp.tile([P, HO, P], BF16, tag="xt", name="xt_v0")
            nc.sync.dma_start(xt_v0[:], d_xt["v"].ap()[:, 0, :])
            for _ko in range(1, HO):
                nc.sync.dma_start(w_v0[:, _ko, :], d_w["v"].ap()[:, _ko, :])

            cos_t = cpool.tile([P, SO, HD], BF16, name="cos_t")
            sin_t = cpool.tile([P, SO, HD], BF16, name="sin_t")
            nc.sync.dma_start(cos_t[:], d_cos.ap())
            nc.sync.dma_start(sin_t[:], d_sin.ap())
            ones = cpool.tile([P, 1], F32, name="ones")
            nc.vector.memset(ones[:], 1.0)
            eps_t = cpool.tile([P, 1], F32, name="eps_t")
            nc.vector.memset(eps_t[:], LN_EPS)
            sqwarm = cpool.tile([P, 1], F32, name="sqwarm")
            nc.scalar.activation(sqwarm[:], eps_t[:],
                                 mybir.ActivationFunctionType.Sqrt)
            # stats rhs: 1/H so the PSUM accumulators hold means directly
            ones_st = cpool.tile([P, 1], BF16, name="ones_st")
            nc.vector.memset(ones_st[:], 1.0 / H)

            def _rope_chunk(a_nat, so):
                """In-place bf16 RoPE on a_nat[:, so, :] (DVE 2x mode)."""
                ch = a_nat[:, so, :]
                ch3 = ch.rearrange("p (nh d) -> p nh d", d=HD)
                ch4 = ch.rearrange("p (nh hf dd) -> p nh hf dd", hf=2, dd=HD // 2)
                rot = rotp.tile([P, H], BF16, tag="rot")
                rot4 = rot[:].rearrange("p (nh hf dd) -> p nh hf dd",
                                        hf=2, dd=HD // 2)
                rot3 = rot[:].rearrange("p (nh d) -> p nh d", d=HD)
                nc.vector.tensor_scalar_mul(rot4[:, :, 0, :], ch4[:, :, 1, :], -1.0)
                nc.vector.tensor_copy(rot4[:, :, 1, :], ch4[:, :, 0, :])
                cb = cos_t[:, so, :][:, None, :].to_broadcast((P, NH, HD))
                sb = sin_t[:, so, :][:, None, :].to_broadcast((P, NH, HD))
                nc.vector.tensor_tensor(ch3[:], ch3[:], cb, mybir.AluOpType.mult)
                nc.vector.tensor_tensor(rot3[:], rot3[:], sb, mybir.AluOpType.mult)
                nc.vector.tensor_tensor(ch[:], ch[:], rot[:], mybir.AluOpType.add)

            def project(a, do_rope=False, w_pre=None, xt0=None):
                """A_nat [P, SO, H] (bf16) = X @ W_a + b_a, optional fused RoPE."""
                if w_pre is None:
                    w_t = big.tile([P, HO, H], BF16, tag="big")
                    for ko in range(HO):
                        nc.sync.dma_start(w_t[:, ko, :], d_w[a].ap()[:, ko, :])
                else:
                    w_t = w_pre
                brow = s4.tile([1, H], F32, tag="s4")
                nc.sync.dma_start(brow[:], d_b[a].ap())
                bb = s4.tile([P, H], F32, tag="s4")
                nc.gpsimd.partition_broadcast(bb[:], brow[:])
                a_nat = big.tile([P, SO, H], BF16, tag="big")
                for so in range(SO):
                    if so == 0 and xt0 is not None:
                        xt_c = xt0
                    else:
                        xt_c = xtp.tile([P, HO, P], BF16, tag="xt")
                        nc.sync.dma_start(xt_c[:], d_xt[a].ap()[:, so, :])
                    for nh in range(2):
                        ps = mmps.tile([P, 512], F32, tag="mm")
                        for ko in range(HO):
                            nc.tensor.matmul(
                                ps[:], xt_c[:, ko, :],
                                w_t[:, ko, nh * 512:(nh + 1) * 512],
                                start=(ko == 0), stop=(ko == HO - 1))
                        nc.vector.tensor_tensor(
                            a_nat[:, so, nh * 512:(nh + 1) * 512], ps[:],
                            bb[:, nh * 512:(nh + 1) * 512], mybir.AluOpType.add)
                    if do_rope:
                        _rope_chunk(a_nat, so)
                return a_nat

            BAND = 12  # T[s', s] == 0 for |s' - s| > 11 (structural)

            def _band_sos(o0, o1):
                """so chunks whose s-range intersects [o0-BAND, o1+BAND)."""
                return [so for so in range(SO)
                        if so * P + P > o0 - BAND and so * P < o1 + BAND]

            def spill_chunk(a_nat, tt, hc, d_scr):
                """One h'-chunk of (T @ A).T spilled to DRAM scratch. Runs
                interleaved with attention, so the eviction rides the DVE
                (the scalar engine is saturated by the Exp stream there)."""
                for sh in range(2):
                    ps = mmps.tile([P, 512], F32, tag="mm")
                    for q in range(2):
                        o0 = sh * 512 + q * 256
                        sos = _band_sos(o0, o0 + 256)
                        for so in sos:
                            nc.tensor.matmul(
                                ps[:, q * 256:(q + 1) * 256],
                                a_nat[:, so, hc * P:(hc + 1) * P],
                                tt[:, so, o0:o0 + 256],
                                start=(so == sos[0]), stop=(so == sos[-1]),
                                skip_group_check=True)
                    ev = s2.tile([P, 512], BF16, tag="s2")
                    nc.vector.tensor_copy(ev[:], ps[:])
                    nc.sync.dma_start(
                        d_scr[hc].ap()[:, sh * 512:(sh + 1) * 512], ev[:])

            def t_agg_v(v_nat, tt):
                """V_ext [P, SO, NH, HD+1] (bf16) = T @ V with ones column."""
                v_ext = big.tile([P, SO, NH, HD + 1], BF16, tag="big")
                nc.vector.tensor_copy(
                    v_ext[:, :, :, HD:HD + 1],
                    ones[:, None, None, :].to_broadcast((P, SO, NH, 1)))
                for sc in range(SO):
                    sos = _band_sos(sc * P, (sc + 1) * P)
                    for dh in range(2):
                        ps = mmps.tile([P, 512], F32, tag="mm")
                        for so in sos:
                            nc.tensor.matmul(
                                ps[:], tt[:, so, sc * P:(sc + 1) * P],
                                v_nat[:, so, dh * 512:(dh + 1) * 512],
                                start=(so == sos[0]), stop=(so == sos[-1]))
                        pvw = ps[:].rearrange("p (nh d) -> p nh d", d=HD)
                        nc.scalar.copy(
                            v_ext[:, sc, dh * 8:(dh + 1) * 8, 0:HD], pvw)
                return v_ext

            # ---- phase 1: V, Q, K  (projection + RoPE + temporal aggregation)
            v_nat = project("v", w_pre=w_v0, xt0=xt_v0)
            tt = big.tile([P, SO, S], BF16, tag="big")
            nc.sync.dma_start(tt[:], d_tt.ap())
            v_ext = t_agg_v(v_nat, tt)

            q_nat = project("q", do_rope=True)
            k_nat = project("k", do_rope=True)

            pre_kq = {}
            # six persistent reload buffers: rows 64:128 are zeroed once and
            # never rewritten (reload DMAs only touch rows 0:64), so the
            # K=128 zero-padding costs no per-head DMA traffic
            kq_bufs = [kqp.tile([P, S], BF16, tag="kq", name=f"kqb{i}")
                       for i in range(6)]
            for b in kq_bufs:
                nc.vector.memset(b[HD:P, :], 0.0)

            def _load_head(h):
                off = (h % 2) * HD
                hc = h // 2
                kh = kq_bufs[(2 * h) % 6]
                nc.sync.dma_start(kh[0:HD, :], d_ks[hc].ap()[off:off + HD, :])
                qh = kq_bufs[(2 * h + 1) % 6]
                nc.sync.dma_start(qh[0:HD, :], d_qs[hc].ap()[off:off + HD, :])
                return kh, qh

            # spill the first two h'-chunks of Q/K aggregation upfront; the
            # remaining chunks interleave with the head loop two chunks ahead
            for hc0 in (0, 1):
                spill_chunk(q_nat, tt, hc0, d_qs)
                spill_chunk(k_nat, tt, hc0, d_ks)
            pre_kq[0] = _load_head(0)
            pre_kq[1] = _load_head(1)

            # prefetch out-projection weights + folded LN rows during phase 2
            wo_t = big.tile([P, HO, H], BF16, tag="big")
            nc.sync.dma_start(wo_t[:], d_w["o"].ap())
            b1r = s4.tile([1, H], F32, tag="s4")
            nc.sync.dma_start(b1r[:], d_b["o"].ap())
            b1b = cpool.tile([P, H], F32, name="b1b")
            nc.gpsimd.partition_broadcast(b1b[:], b1r[:])
            g1r = s4.tile([1, H], F32, tag="s4")
            nc.sync.dma_start(g1r[:], d_g1n.ap())
            g1nb = cpool.tile([P, H], F32, name="g1nb")
            nc.gpsimd.partition_broadcast(g1nb[:], g1r[:])

            # ---- phase 2: attention. One attention-out tile per h' chunk
            # (separate tensors keep the scheduler from inventing cross-chunk
            # dependencies); LN stats accumulate on the DVE only.
            attn_c = [atp.tile([P, S], BF16, tag="at", name=f"attn{c}")
                      for c in range(HO)]
            acc = accp.tile([P, S], BF16, tag="acc", name="acc")
            acc2 = accp.tile([P, S], BF16, tag="acc", name="acc2")

            rb_c = None
            for h in range(NH):
                hc, off = h // 2, (h % 2) * HD
                # zero-pad the contraction dim to K=128 (rows 64:128 from a
                # DRAM zeros pad) to keep the PE activity profile flat
                if h in pre_kq:
                    kh, qh = pre_kq[h]
                else:
                    kh, qh = _load_head(h)
                if off == 0:
                    if hc + 2 < HO:
                        spill_chunk(q_nat, tt, hc + 2, d_qs)
                        spill_chunk(k_nat, tt, hc + 2, d_ks)
                    rb_c = s4.tile([P, S], F32, tag="s4")
                    rcp_c = s4.tile([P, S], F32, tag="s4")
                else:
                    # even head's rows normalize while the odd head computes,
                    # halving the serial chain at every chunk boundary. The
                    # custom reciprocal op must start at partition 0 (its
                    # seed constants are partition-aligned), so it runs on
                    # the full tile; the top half is recomputed at the
                    # boundary once the odd head's denominators land.
                    nc.vector.reciprocal_approx_fast(rcp_c[:], rb_c[:])
                    nc.vector.tensor_tensor(attn_c[hc][0:HD, :],
                                            attn_c[hc][0:HD, :],
                                            rcp_c[0:HD, :],
                                            mybir.AluOpType.mult)
                pv_a = pvps.tile([P, 512], F32, tag="pv", name="pv_a")
                pv_b = pvps.tile([P, 512], F32, tag="pv", name="pv_b")
                # software-pipelined: both query halves of a key chunk land in
                # one two-bank PSUM tile so a single Exp serves the chunk,
                # running AHEAD chunks in front of the PV accumulation
                ets = []
                for kc in range(SO + AHEAD):
                    if kc < SO:
                        sp = scps.tile([P, 1024], F32, tag="sc")
                        nc.tensor.matmul(
                            sp[:, 0:512], kh[0:P, kc * P:(kc + 1) * P],
                            qh[0:P, 0:512],
                            start=True, stop=True, skip_group_check=True)
                        nc.tensor.matmul(
                            sp[:, 512:1024], kh[0:P, kc * P:(kc + 1) * P],
                            qh[0:P, 512:1024],
                            start=True, stop=True, skip_group_check=True)
                        e_t = etp.tile([P, 1024], BF16, tag="et")
                        nc.scalar.activation(
                            e_t[:], sp[:],
                            mybir.ActivationFunctionType.Exp, scale=0.125)
                        ets.append(e_t)
                    if kc >= AHEAD:
                        j = kc - AHEAD
                        nc.tensor.matmul(
                            pv_a[0:HD + 1, :], v_ext[:, j, h, :],
                            ets[j][:, 0:512],
                            start=(j == 0), stop=(j == SO - 1),
                            skip_group_check=True)
                        nc.tensor.matmul(
                            pv_b[0:HD + 1, :], v_ext[:, j, h, :],
                            ets[j][:, 512:1024],
                            start=(j == 0), stop=(j == SO - 1),
                            skip_group_check=True)
                # evict raw out + sums; broadcast sums (no PE dependency).
                # partition_broadcast only writes reliably at partition 0,
                # so odd heads bounce through a temp + DVE copy.
                for q2, pv in ((0, pv_a), (1, pv_b)):
                    qs = slice(q2 * 512, (q2 + 1) * 512)
                    nc.vector.tensor_copy(attn_c[hc][off:off + HD, qs],
                                          pv[0:HD, :])
                    srow = s2.tile([1, 512], F32, tag="s2")
                    nc.vector.tensor_copy(srow[:], pv[HD:HD + 1, :])
                    if off == 0:
                        nc.gpsimd.partition_broadcast(rb_c[0:HD, qs], srow[:])
                    else:
                        tmp = s2.tile([HD, 512], F32, tag="s2")
                        nc.gpsimd.partition_broadcast(tmp[:], srow[:])
                        nc.vector.tensor_copy(rb_c[off:off + HD, qs], tmp[:])
                if off == HD:
                    # chunk hc complete: odd head's rows + LN stats (all DVE)
                    rcp2 = s4.tile([P, S], F32, tag="s4")
                    nc.vector.reciprocal_approx_fast(rcp2[:], rb_c[:])
                    nc.vector.tensor_tensor(attn_c[hc][HD:P, :],
                                            attn_c[hc][HD:P, :],
                                            rcp2[HD:P, :],
                                            mybir.AluOpType.mult)
                    sq_c = sqp.tile([P, S], BF16, tag="sq")
                    nc.vector.tensor_tensor(sq_c[:], attn_c[hc][:],
                                            attn_c[hc][:],
                                            mybir.AluOpType.mult)
                    if hc == 0:
                        nc.vector.tensor_copy(acc[:], attn_c[0][:])
                        nc.vector.tensor_copy(acc2[:], sq_c[:])
                    elif hc < HO - 1:
                        nc.vector.tensor_tensor(acc[:], acc[:], attn_c[hc][:],
                                                mybir.AluOpType.add)
                        nc.vector.tensor_tensor(acc2[:], acc2[:], sq_c[:],
                                                mybir.AluOpType.add)
                    else:
                        sq7 = sq_c  # last chunk's stats go straight to PSUM

            # out-proj partial chains for the first two (so, nh) pairs:
            # chunks 0..6 are ready one boundary early, so these fill the PE
            # while the last chunk's normalize chain runs on the DVE
            pairs = [(so, nh) for so in range(SO) for nh in range(2)]
            pair_ps = []
            for so, nh in pairs[:2]:
                psp = mmps.tile([P, 512], F32, tag="mm", name="psp")
                pair_ps.append(psp)
                for hc in range(HO - 1):
                    nc.tensor.matmul(
                        psp[:], attn_c[hc][:, so * P:(so + 1) * P],
                        wo_t[:, hc, nh * 512:(nh + 1) * 512],
                        start=(hc == 0), stop=False)

            # ---- phase 3: one block of transposed stats matmuls ([s-part,
            # so] layout) + LN scale factors -- all tiny ops
            stat_t = pvps.tile([P, 32], F32, tag="pv", name="stat_t")
            for so in range(SO):
                # chunks 0..6 ride acc/acc2 (hoistable off the tail); chunk 7
                # feeds its own columns directly so only these 16 tiny
                # matmuls trail the last normalize
                nc.tensor.matmul(
                    stat_t[:, so:so + 1], acc[:, so * P:(so + 1) * P],
                    ones_st[:], start=True, stop=True, skip_group_check=True)
                nc.tensor.matmul(
                    stat_t[:, 8 + so:9 + so], acc2[:, so * P:(so + 1) * P],
                    ones_st[:], start=True, stop=True, skip_group_check=True)
                nc.tensor.matmul(
                    stat_t[:, 16 + so:17 + so],
                    attn_c[HO - 1][:, so * P:(so + 1) * P],
                    ones_st[:], start=True, stop=True, skip_group_check=True)
                nc.tensor.matmul(
                    stat_t[:, 24 + so:25 + so], sq7[:, so * P:(so + 1) * P],
                    ones_st[:], start=True, stop=True, skip_group_check=True)
            acc16 = s2.tile([P, 16], F32, tag="s2")
            nc.vector.tensor_copy(acc16[:], stat_t[:, 0:16])
            nc.vector.tensor_tensor(acc16[:], acc16[:], stat_t[:, 16:32],
                                    mybir.AluOpType.add)
            m2 = s2.tile([P, 8], F32, tag="s2")
            nc.scalar.square(m2[:], acc16[:, 0:8])
            var_t = s2.tile([P, 8], F32, tag="s2")
            nc.vector.tensor_tensor(var_t[:], acc16[:, 8:16], m2[:],
                                    mybir.AluOpType.subtract)
            nc.scalar.activation(var_t[:], var_t[:],
                                 mybir.ActivationFunctionType.Sqrt, bias=eps_t[:])
            rstd = s2.tile([P, 8], F32, tag="s2")
            nc.vector.reciprocal_approx_fast(rstd[:], var_t[:])
            rmu = s2.tile([P, 8], F32, tag="s2")
            nc.vector.tensor_tensor(rmu[:], acc16[:, 0:8], rstd[:],
                                    mybir.AluOpType.mult)

            # ---- phase 4: output projection on raw attn with fused LN:
            # y = rstd*(attn^T @ W') - (mu*rstd)*g1 + b1. The first two
            # pairs' chunk-0..6 partial chains were issued before the stats
            # block (above), filling the PE while the last normalize runs.
            for i, (so, nh) in enumerate(pairs):
                ps = pair_ps[i] if i < 2 else mmps.tile([P, 512], F32,
                                                        tag="mm", name="ps")
                h0 = HO - 1 if i < 2 else 0
                for hc in range(h0, HO):
                    nc.tensor.matmul(
                        ps[:], attn_c[hc][:, so * P:(so + 1) * P],
                        wo_t[:, hc, nh * 512:(nh + 1) * 512],
                        start=(hc == 0), stop=(hc == HO - 1))
                t2 = s2.tile([P, 512], F32, tag="s2")
                nc.vector.scalar_tensor_tensor(
                    t2[:], g1nb[:, nh * 512:(nh + 1) * 512],
                    rmu[:, so:so + 1], b1b[:, nh * 512:(nh + 1) * 512],
                    mybir.AluOpType.mult, mybir.AluOpType.add)
                ych = s2.tile([P, 512], F32, tag="s2")
                nc.vector.scalar_tensor_tensor(
                    ych[:], ps[:], rstd[:, so:so + 1], t2[:],
                    mybir.AluOpType.mult, mybir.AluOpType.add)
                nc.sync.dma_start(
                    d_y.ap()[:, so, nh * 512:(nh + 1) * 512], ych[:])

    nc.compile()
    return nc


_NC = None


def _get_nc():
    global _NC
    if _NC is None:
        _NC = _build_program()
    return _NC


def _host_inputs(query, key, value, Wq, bq, Wk, bk, Wv, bv, Wo, bo,
                 temporal_weights, ln_gamma, ln_beta):
    T = _temporal_matrix(temporal_weights)
    tt_host = np.ascontiguousarray(  # TT[p, so, s'] = T[s', so*P+p]
        T.T.reshape(SO, P, S).transpose(1, 0, 2)).astype(NPBF16)
    cos, sin = _rope_tables()
    gam = np.asarray(ln_gamma, np.float32)
    bet = np.asarray(ln_beta, np.float32)
    Wo32 = np.asarray(Wo, np.float32)
    Wfold = gam[:, None] * Wo32               # gamma folded into out-proj
    g1n = -(gam @ Wo32).reshape(1, H)         # rank-1 LN correction row
    b1 = (bet @ Wo32 + np.asarray(bo, np.float32)).reshape(1, H)
    common = {
        "w_v": _nat(np.asarray(Wv, np.float32)).astype(NPBF16),
        "w_q": _nat(np.asarray(Wq, np.float32)).astype(NPBF16),
        "w_k": _nat(np.asarray(Wk, np.float32)).astype(NPBF16),
        "w_o": _nat(Wfold).astype(NPBF16),
        "b_v": np.asarray(bv, np.float32).reshape(1, H),
        "b_q": np.asarray(bq, np.float32).reshape(1, H),
        "b_k": np.asarray(bk, np.float32).reshape(1, H),
        "b_o": b1,
        "g1n": g1n,
        "tt": tt_host,
        "zpad": np.zeros((HD, S), NPBF16),
        "cos_t": _nat(cos).astype(NPBF16),
        "sin_t": _nat(sin).astype(NPBF16),
    }
    in_maps = []
    for c in range(N_CORES):
        m = dict(common)
        m["xt_q"] = _xt_chunks(np.asarray(query[c], np.float32)).astype(NPBF16)
        m["xt_k"] = _xt_chunks(np.asarray(key[c], np.float32)).astype(NPBF16)
        m["xt_v"] = _xt_chunks(np.asarray(value[c], np.float32)).astype(NPBF16)
        in_maps.append(m)
    return in_maps


def kernel(query, key, value, Wq, bq, Wk, bk, Wv, bv, Wo, bo,
           temporal_weights, ln_gamma, ln_beta):
    in_maps = _host_inputs(query, key, value, Wq, bq, Wk, bk, Wv, bv, Wo, bo,
                           temporal_weights, ln_gamma, ln_beta)
    nc = _get_nc()
    res = run_bass_kernel_spmd(nc, in_maps, list(range(N_CORES)))
    out = np.empty((B, S, H), np.float32)
    for c in range(N_CORES):
        y = res.results[c]["y"]  # [P, SO, H]
        out[c] = y.transpose(1, 0, 2).reshape(S, H)
    return out
